# revision 32
# baseline (speedup 1.0000x reference)
"""Trainium2 Bass kernel for BiasedAxialAttention (tied row attention), 8-core SPMD.

Math (reference, in the transposed frame X = LN(pairT), pairT[a,b,:] = pair[0,b,a,:]):
    q,k,v = X@Wq,Wk,Wv (scaled);  b = LN(biasT)@Wb
    g = sigmoid(X@Wg + bg)
    logits[i,j,h] = sum_{n,d} q[n,i,h,d] k[n,j,h,d] + b[i,j,h]
    attn = softmax_j(logits);  out[i,k,(h,d)] = sum_j attn[i,j,h] v[k,j,h,d]
    out = (g * out) @ Wo + bo;  final[k,i,:] = out[i,k,:]

Sharding (core c of 8): rows R_c = [32c, 32c+32) of X are both the tied
contraction rows (n) and the core's output columns (k). Partial logits are
AllReduduced in fp16 (the bias b is folded into the partials pre-AR, so no
AllGather is needed).

Host ships activations fp16, channel-major (pre-transposed):
    pr[chan, r*2048 + g*256 + i] = X_raw[32c + 4g + r, i, chan]
    pc[chan, g*1024 + r*256 + i] = X_raw[i, 32c + 4g + r, chan]
    br[chan, s*256 + j]          = biasT_raw[32c + s, j, chan]
LN runs on-device: per-token sums via ones-matmuls (replicated over 32-row
PSUM groups via tile_position), finishing on a [128,64] reshape, then the
(r | -m*r) row is partition-broadcast (GPSIMD) and applied as two
tensor_tensor passes. LN gamma and all static scales are folded into the
weights on the host; LN beta must be zero and the mask all-ones (asserted).
"""
import os
import sys

for _p in ("/opt/trn_rl_repo", "/root/.axon_site/_ro/trn_rl_repo"):
    if os.path.isdir(_p) and _p not in sys.path:
        sys.path.append(_p)

import math
import numpy as np

N_CORES = 8
L = 256
D = 128
H = 8
DH = 32
HD = H * DH          # 256
NS = L // N_CORES    # 32
NTOK = NS * L        # 8192
EPS = 1e-5

_cache = {}


def _build():
    if "nc" in _cache:
        return _cache["nc"]
    from contextlib import ExitStack

    import concourse.bacc as bacc
    import concourse.bass as cbass
    import concourse.tile as tile
    from concourse import mybir

    F32 = mybir.dt.float32
    F16 = mybir.dt.float16
    AF = mybir.ActivationFunctionType
    ALU = mybir.AluOpType

    nc = bacc.Bacc("TRN2", target_bir_lowering=False, debug=False,
                   num_devices=N_CORES)

    ei = dict(kind="ExternalInput")
    pr_d = nc.dram_tensor("pr", [D, NTOK], F16, **ei)
    pc_d = nc.dram_tensor("pc", [D, NTOK], F16, **ei)
    br_d = nc.dram_tensor("br", [D, NTOK], F16, **ei)
    # w_all: [D, 4*HD + 32 + 32]: wq|wk|wv|wg|wb_x4|ones32 (gamma, scales folded)
    WCOL = 4 * HD + 32 + 32
    wall_d = nc.dram_tensor("w_all", [D, WCOL], F16, **ei)
    wo2_d = nc.dram_tensor("wo2", [128, 2 * D], F16, **ei)
    bo_d = nc.dram_tensor("bo", [D, 1], F32, **ei)
    bg_d = nc.dram_tensor("bg2", [128, 2], F32, **ei)
    id_d = nc.dram_tensor("ident", [128, 128], F16, **ei)

    out_d = nc.dram_tensor("out", [D, NTOK], F16, kind="ExternalOutput")

    with tile.TileContext(nc) as tc, ExitStack() as ctx:
        singles = ctx.enter_context(tc.tile_pool(name="singles", bufs=1))
        small = ctx.enter_context(tc.tile_pool(name="small", bufs=1))
        p2 = ctx.enter_context(tc.tile_pool(name="p2", bufs=2))
        big = ctx.enter_context(tc.tile_pool(name="big", bufs=1))
        ps = ctx.enter_context(tc.tile_pool(name="ps", bufs=2, space="PSUM"))
        ps_st = ctx.enter_context(tc.tile_pool(name="ps_st", bufs=1,
                                               space="PSUM"))
        ps_t = ctx.enter_context(tc.tile_pool(name="ps_t", bufs=2,
                                              space="PSUM"))
        dram = ctx.enter_context(tc.tile_pool(name="dram", bufs=1,
                                              space="DRAM"))

        # ---------------- constants ----------------
        w_sb = singles.tile([128, WCOL], F16, tag="w_sb")
        nc.scalar.dma_start(w_sb[:], wall_d.ap())
        wq = w_sb[:, 0:HD]
        wk = w_sb[:, HD:2 * HD]
        wv = w_sb[:, 2 * HD:3 * HD]
        wg = w_sb[:, 3 * HD:4 * HD]
        wb = w_sb[:, 4 * HD:4 * HD + 32]
        ones32 = w_sb[:, 4 * HD + 32:4 * HD + 64]
        wo_sb = singles.tile([128, 2 * D], F16, tag="wo_sb")
        nc.scalar.dma_start(wo_sb[:], wo2_d.ap())
        bo_sb = singles.tile([128, 1], F32, tag="bo")
        nc.scalar.dma_start(bo_sb[:], bo_d.ap())
        bg_sb = singles.tile([128, 2], F32, tag="bg")
        nc.scalar.dma_start(bg_sb[:], bg_d.ap())
        ident = singles.tile([128, 128], F16, tag="ident")
        nc.scalar.dma_start(ident[:], id_d.ap())
        eps_t = singles.tile([128, 1], F32, tag="eps")
        nc.vector.memset(eps_t[:], EPS)

        # ---------------- input loads (4 chunks each) -----------------------
        xt_br = big.tile([128, NTOK], F16, tag="xt_br")
        xt_pr = big.tile([128, NTOK], F16, tag="xt_pr")
        xt_pc = big.tile([128, NTOK], F16, tag="xt_pc")
        for t4 in range(4):
            sl = slice(2048 * t4, 2048 * (t4 + 1))
            nc.sync.dma_start(xt_br[:, sl], br_d.ap()[:, sl])
        for t4 in range(4):
            sl = slice(2048 * t4, 2048 * (t4 + 1))
            nc.sync.dma_start(xt_pr[:, sl], pr_d.ap()[:, sl])
        for t4 in range(4):
            sl = slice(2048 * t4, 2048 * (t4 + 1))
            nc.scalar.dma_start(xt_pc[:, sl], pc_d.ap()[:, sl])

        # ---------------- DRAM collective tiles -----------------------------
        ar_in = [dram.tile([4, 2, 128, L], F16, tag=f"ar_in{i}",
                           name=f"ar_in{i}") for i in range(2)]
        ar_out = [dram.tile([4, 2, 128, L], F16, tag=f"ar_out{i}",
                            name=f"ar_out{i}", addr_space="Shared")
                  for i in range(2)]

        # =============== layernorm: stats + normalize ========================
        # stats: Sx(t) and Sq(t) (pre-scaled by 1/128) for all 8192 tokens.
        # ones32-matmul replicates each chunk's sums over a 32-row PSUM group;
        # chunk j -> rows [32*(j//4), +32), cols [512*(j%4), +512).
        def ln_stats(xt, nm):
            sq = big.tile([128, NTOK], F16, tag="ppq", name="sq")
            for blk in range(4):
                nc.vector.tensor_tensor(
                    out=sq[:, 2048 * blk:2048 * (blk + 1)],
                    in0=xt[:, 2048 * blk:2048 * (blk + 1)],
                    in1=xt[:, 2048 * blk:2048 * (blk + 1)], op=ALU.mult)
            sx4 = small.tile([128, 2048], F16, tag="sx4")
            sq4 = small.tile([128, 2048], F16, tag="sx4", name="sq4")
            st_d = [dram.tile([4, 2048], F16, tag=f"st_d{i}",
                              name=f"st_d{i}") for i in range(2)]
            for st in range(2):
                mps = ps_st.tile([128, 2048], F32, tag="stat",
                                 name=f"mps{st}")
                srcbuf = xt if st == 0 else sq
                for j in range(16):   # 512-token chunks
                    rg, slot = j // 4, j % 4
                    nc.tensor.matmul(
                        mps[32 * rg:32 * (rg + 1),
                            512 * slot:512 * (slot + 1)], ones32,
                        srcbuf[:, 512 * j:512 * (j + 1)],
                        start=True, stop=True, tile_position=(0, 32 * rg))
                # evac full replicated tile (engine partition access must
                # be contiguous); the DMA then picks rows {0,32,64,96}.
                dst = sx4 if st == 0 else sq4
                if st == 0:
                    nc.vector.tensor_copy(dst[:], mps[:])
                else:
                    nc.scalar.activation(out=dst[:], in_=mps[:], func=AF.Copy)
                nc.sync.dma_start(
                    st_d[st][:],
                    dst[:].rearrange("(a b) f -> a b f", b=32)[:, 0, :])
            # [128, 64] views: token t = 64*p + f
            m128 = small.tile([128, 64], F16, tag="m128")
            e128 = small.tile([128, 64], F16, tag="e128")
            nc.sync.dma_start(
                m128[:], st_d[0][:].rearrange("a (b f) -> (a b) f", f=64))
            nc.sync.dma_start(
                e128[:], st_d[1][:].rearrange("a (b f) -> (a b) f", f=64))
            # finishing: var = E[x^2] - m^2 ; r = 1/sqrt(var+eps); nmr = -m*r
            var = small.tile([128, 64], F16, tag="var")
            nc.vector.scalar_tensor_tensor(
                out=var[:], in0=m128[:], scalar=-1.0, in1=m128[:],
                op0=ALU.mult, op1=ALU.mult)            # -m^2
            nc.vector.tensor_tensor(out=var[:], in0=e128[:], in1=var[:],
                                    op=ALU.add)        # E[x^2]-m^2
            std = small.tile([128, 64], F32, tag="std")
            nc.scalar.activation(out=std[:], in_=var[:], func=AF.Sqrt,
                                 bias=eps_t[:], scale=1.0)
            rec = small.tile([128, 64], F32, tag="rec")
            nc.vector.reciprocal(out=rec[:], in_=std[:])
            r16 = small.tile([128, 64], F16, tag="r16")
            nc.vector.tensor_copy(r16[:], rec[:])
            nmr = small.tile([128, 64], F16, tag="nmr")
            nc.vector.scalar_tensor_tensor(
                out=nmr[:], in0=m128[:], scalar=-1.0, in1=rec[:],
                op0=ALU.mult, op1=ALU.mult)
            # rows [1, 8192] each via DRAM bounce; t = 64*p + f
            rm_d = dram.tile([2, 128, 64], F16, tag=f"rm_d_{nm}",
                             name=f"rm_d_{nm}")
            nc.sync.dma_start(rm_d[0], r16[:])
            nc.sync.dma_start(rm_d[1], nmr[:])
            return rm_d

        def ln_norm(xt, rm_d):
            # broadcast r / -m*r rows from DRAM (stride-0 partition source)
            # into a [128, NTOK] SBUF tile, applied as two TT passes.
            rm_bc = big.tile([128, NTOK], F16, tag="rm_bc", name="rm_bc")
            nc.scalar.dma_start(
                rm_bc[:],
                rm_d[0].rearrange("p f -> (p f)").partition_broadcast(128))
            for blk in range(4):
                sl = slice(2048 * blk, 2048 * (blk + 1))
                nc.vector.tensor_tensor(out=xt[:, sl], in0=xt[:, sl],
                                        in1=rm_bc[:, sl], op=ALU.mult)
            nc.scalar.dma_start(
                rm_bc[:],
                rm_d[1].rearrange("p f -> (p f)").partition_broadcast(128))
            for blk in range(4):
                sl = slice(2048 * blk, 2048 * (blk + 1))
                nc.vector.tensor_tensor(out=xt[:, sl], in0=xt[:, sl],
                                        in1=rm_bc[:, sl], op=ALU.add)

        rm_br = ln_stats(xt_br, "br")
        rm_pr = ln_stats(xt_pr, "pr")
        rm_pc = ln_stats(xt_pc, "pc")
        ln_norm(xt_br, rm_br)
        ln_norm(xt_pr, rm_pr)

        # ====== b projection -> b_d DRAM [4 t4, 8 h, (8 s8, 256 j)] =========
        # b[h, s = 8*t4 + s8, j] lives at b_d[t4, h, (s8, j)]; the pre-AR
        # accumulate DMAs read it DRAM->DRAM with the CCE add.
        b_d = dram.tile([4, 8, 2048], F16, tag="b_d", name="b_d")
        for t4 in range(4):
            pb = ps_st.tile([32, 2048], F32, tag="stat", name="b_ps")
            for j in range(4):
                nc.tensor.matmul(pb[:, 512 * j:512 * (j + 1)], wb,
                                 xt_br[:, 2048 * t4 + 512 * j:
                                       2048 * t4 + 512 * (j + 1)],
                                 start=True, stop=True)
            bh = p2.tile([8, 2048], F16, tag="b_hd", name=f"bh{t4}")
            if t4 % 2 == 0:
                nc.vector.tensor_copy(bh[:], pb[0:8, :])
            else:
                nc.scalar.activation(out=bh[:], in_=pb[0:8, :], func=AF.Copy)
            nc.scalar.dma_start(b_d[t4], bh[:])

        # =============== q/k projection + pack + logits per half =============
        ppq = big.tile([128, NTOK], F16, tag="ppq")
        ppk = big.tile([128, NTOK], F16, tag="ppk")
        pkq_all = big.tile([128, 4, 8, L], F16, tag="xt_br", name="pkq_all")
        pkk_all = big.tile([128, 4, 8, L], F16, tag="pkk_all",
                           name="pkk_all")
        pkq = [pkq_all[:, i] for i in range(4)]
        pkk = [pkk_all[:, i] for i in range(4)]

        def evac(dst, src, k):
            if k % 3 in (0, 1):
                nc.scalar.activation(out=dst, in_=src, func=AF.Copy)
            else:
                nc.vector.tensor_copy(dst, src)

        def proj_half(w_, half, dst):
            for ch in range(16):
                pm = ps.tile([128, 512], F32, tag="mm")
                nc.tensor.matmul(pm[:], w_[:, 128 * half:128 * (half + 1)],
                                 dst[1][:, 512 * ch:512 * (ch + 1)],
                                 start=True, stop=True)
                evac(dst[0][:, 512 * ch:512 * (ch + 1)], pm[:], ch)

        ls_stage = [p2.tile([128, 4, 2, L], F16, tag="lss", name=f"lss{i}")
                    for i in range(2)]

        def qk_half(half):
            proj_half(wq, half, (ppq, xt_pr))
            proj_half(wk, half, (ppk, xt_pr))
            for hq in range(4):
                for r in range(4):
                    nc.sync.dma_start(
                        pkq[hq][32 * r:32 * (r + 1), :, :]
                        .rearrange("d g i -> d (g i)"),
                        ppq[32 * hq:32 * (hq + 1),
                            2048 * r:2048 * (r + 1)])
                    nc.scalar.dma_start(
                        pkk[hq][32 * r:32 * (r + 1), :, :]
                        .rearrange("d g i -> d (g i)"),
                        ppk[32 * hq:32 * (hq + 1),
                            2048 * r:2048 * (r + 1)])
            lss = ls_stage[half]
            for hq in range(4):
                pl = ps.tile([128, 2, L], F32, tag="mm", name="pl")
                for ih in range(2):
                    for g in range(8):
                        nc.tensor.matmul(
                            pl[:, ih, :],
                            pkq[hq][:, g, 128 * ih:128 * (ih + 1)],
                            pkk[hq][:, g, :], start=(g == 0), stop=(g == 7))
                evac(lss[:, hq, :, :].rearrange("p a b -> p (a b)"),
                     pl[:].rearrange("p a b -> p (a b)"), hq)
            return lss

        # fold b into the partial logits pre-AR: core c owns global rows
        # i = 32*(c%4) + s in half ih_c = c//4; DMA-accumulate b_t into
        # ar_in at a core-id-dependent DRAM offset (CCE add on the DMA).
        if not os.environ.get("KNOB_NO_BADD"):
            pid = nc.gpsimd.partition_id()
            ih_reg = pid // 4
            pc_reg = (pid % 4) * 32

        def run_half(half):
            lss = qk_half(half)
            nc.sync.dma_start(
                ar_in[half][:].rearrange("hq ih p j -> p hq ih j"), lss[:])
            if not os.environ.get("KNOB_NO_BADD"):
                for t4 in range(4):
                    nc.gpsimd.dma_start(
                        ar_in[half][:, cbass.ds(ih_reg, 1),
                                    cbass.ds(pc_reg + 8 * t4, 8), :],
                        b_d[t4, 4 * half:4 * (half + 1), :]
                        .rearrange("h (s j) -> h s j", s=8),
                        accum_op=ALU.add)
            if os.environ.get("KNOB_NO_AR"):
                nc.scalar.dma_start(ar_out[half][:], ar_in[half][:])
            else:
                nc.gpsimd.collective_compute(
                    "AllReduce", ALU.add,
                    replica_groups=[list(range(N_CORES))],
                    ins=[ar_in[half].opt()], outs=[ar_out[half].opt()],
                )

        run_half(0)
        ln_norm(xt_pc, rm_pc)
        run_half(1)

        # =============== v projection -> vkg[g] ==============================
        vkg_all = big.tile([128, 8, 2, 8, 4, DH], F16, tag="vkg_all",
                           name="vkg_all")
        vkg = [vkg_all[:, g] for g in range(8)]
        kv = 0
        for g in range(8):
            for r in range(4):
                for ih in range(2):
                    coff = 2048 * r + 256 * g + 128 * ih
                    pv = ps.tile([128, 256], F32, tag="mm", name="pv")
                    nc.tensor.matmul(pv[:], xt_pr[:, coff:coff + 128], wv,
                                     start=True, stop=True)
                    dst = vkg[g][:, ih, :, r, :]
                    src = pv[:].rearrange("p (h d) -> p h d", h=8)
                    evac(dst, src, kv)
                    kv += 1

        # =============== gate projection (sigmoid) ===========================
        gs = [big.tile([128, NTOK], F16, tag="ppq", name="gs0"),
              big.tile([128, NTOK], F16, tag="ppk", name="gs1")]
        for q in range(2):
            for ch in range(16):
                pg = ps.tile([128, 512], F32, tag="mm", name="pg")
                nc.tensor.matmul(pg[:], wg[:, 128 * q:128 * (q + 1)],
                                 xt_pc[:, 512 * ch:512 * (ch + 1)],
                                 start=True, stop=True)
                nc.scalar.activation(
                    out=gs[q][:, 512 * ch:512 * (ch + 1)], in_=pg[:],
                    func=AF.Sigmoid, bias=bg_sb[:, q:q + 1], scale=1.0)

        # =============== softmax per half -> at_t ============================
        at_t = big.tile([128, 2, 8, 2, 128], F16, tag="xt_br", name="at_t")
        atn = p2.tile([128, 4, 2, L], F16, tag="lss", name="atn")

        def softmax(half):
            lsb = p2.tile([128, 4, 2, L], F16, tag="lsb", bufs=1)
            nc.sync.dma_start(
                lsb[:], ar_out[half][:].rearrange("hq ih p j -> p hq ih j"))
            sm_s = small.tile([128, 8], F32, tag="sm_s")
            nmx = small.tile([128, 8], F32, tag="nmx")
            for hq in range(4):
                for ih in range(2):
                    cc = 2 * hq + ih
                    nc.vector.tensor_reduce(
                        out=nmx[:, cc:cc + 1], in_=lsb[:, hq, ih, :],
                        axis=mybir.AxisListType.X, op=ALU.max, negate=True)
            for hq in range(4):
                for ih in range(2):
                    cc = 2 * hq + ih
                    nc.scalar.activation(
                        out=atn[:, hq, ih, :], in_=lsb[:, hq, ih, :],
                        func=AF.Exp, bias=nmx[:, cc:cc + 1], scale=1.0,
                        accum_out=sm_s[:, cc:cc + 1])
            sm_r = small.tile([128, 8], F32, tag="sm_r")
            nc.vector.reciprocal(out=sm_r[:], in_=sm_s[:])
            for hq in range(4):
                h = 4 * half + hq
                for ih in range(2):
                    nc.vector.tensor_scalar(
                        out=atn[:, hq, ih, :], in0=atn[:, hq, ih, :],
                        scalar1=sm_r[:, 2 * hq + ih:2 * hq + ih + 1],
                        scalar2=None, op0=ALU.mult)
                pt = ps_t.tile([128, 2, 2, 128], F16, tag="pt")
                for ih in range(2):
                    for jh in range(2):
                        nc.tensor.transpose(
                            pt[:, ih, jh, :],
                            atn[:, hq, ih, 128 * jh:128 * (jh + 1)],
                            ident[:])
                evac(at_t[:, :, h, :, :].rearrange("p jh ih i -> p ih jh i"),
                     pt[:], hq)

        softmax(0)

        # ======= einsum phase A (heads 0-3 -> even h' -> okg q=0) ==========
        # runs inside the AR1 shadow; phase B (heads 4-7, q=1) follows
        # softmax(1), then the Wo projection consumes both.
        okg_all = big.tile([128, 8, 2, 4, L], F16, tag="xt_pr",
                           name="okg_all")

        def ein_half(g, hb):
            ein = p2.tile([128, 8, L], F16, tag="ein", name=f"ein{g}_{hb}")
            for hp in (2 * hb, 2 * hb + 1):
                hps = hp if hb == 0 else hp - 2
                po = ps.tile([128, 2, L], F32, tag="mm", name="po")
                for hh in range(2):
                    h = (4 * hb) + 2 * hps + hh if False else 2 * hp + hh
                    for jh in range(2):
                        nc.tensor.matmul(
                            po[:, hh, :],
                            vkg[g][:, jh, h, :, :]
                            .rearrange("p kl d -> p (kl d)"),
                            at_t[:, jh, h, :, :]
                            .rearrange("p ih i -> p (ih i)"),
                            start=(jh == 0), stop=(jh == 1))
                dst = ein[:].rearrange("p (k par) i -> p par k i", par=2)[
                    :, hp // 2, 2 * (hp % 2):2 * (hp % 2) + 2, :]
                evac(dst, po[:], hp + g)
            q = hb
            for kl in range(4):
                eng = nc.sync if kl % 2 == 0 else nc.scalar
                eng.dma_start(
                    okg_all[:, g, q, kl, :],
                    ein[32 * kl:32 * (kl + 1), :, :]
                    .rearrange("d (hq q) i -> d q hq i", q=2)[:, q, :, :])
            nc.vector.tensor_tensor(
                out=okg_all[:, g, q, :, :].rearrange("p kl i -> p (kl i)"),
                in0=okg_all[:, g, q, :, :].rearrange("p kl i -> p (kl i)"),
                in1=gs[q][:, 1024 * g:1024 * (g + 1)], op=ALU.mult)

        def wo_block(g):
            ot = p2.tile([128, 1024], F16, tag="ot", name=f"ot{g}",
                         bufs=1)
            for cc in range(2):
                pw = ps.tile([128, 512], F32, tag="mm", name="pw")
                nc.tensor.matmul(pw[:], wo_sb[:, 0:D],
                                 okg_all[:, g, 0, 2 * cc:2 * (cc + 1), :]
                                 .rearrange("p kl i -> p (kl i)"),
                                 start=True, stop=False)
                nc.tensor.matmul(pw[:], wo_sb[:, D:2 * D],
                                 okg_all[:, g, 1, 2 * cc:2 * (cc + 1), :]
                                 .rearrange("p kl i -> p (kl i)"),
                                 start=False, stop=True)
                if (g + cc) % 2 == 0:
                    nc.scalar.activation(out=ot[:, 512 * cc:512 * (cc + 1)],
                                         in_=pw[:], func=AF.Identity,
                                         bias=bo_sb[:], scale=1.0)
                else:
                    nc.vector.tensor_scalar(
                        out=ot[:, 512 * cc:512 * (cc + 1)], in0=pw[:],
                        scalar1=bo_sb[:], scalar2=None, op0=ALU.add)
            eng = nc.sync if g % 2 == 0 else nc.scalar
            eng.dma_start(out_d.ap()[:, 1024 * g:1024 * (g + 1)], ot[:])

        for g in range(8):
            ein_half(g, 0)
        softmax(1)
        ein_half(0, 1)
        for g in range(1, 8):
            ein_half(g, 1)
            wo_block(g - 1)
        wo_block(7)

    nc.compile()
    _cache["nc"] = nc
    return nc


def _prep_inputs(inputs):
    import ml_dtypes
    F16 = np.float16
    pair = np.asarray(inputs["pair"], dtype=np.float32)
    bias = np.asarray(inputs["bias"], dtype=np.float32)
    mask = np.asarray(inputs["mask"])
    assert bool(mask.all()), "kernel specialized for all-ones mask"
    lnpw = np.asarray(inputs["ln_pair_w"], np.float32)
    lnpb = np.asarray(inputs["ln_pair_b"], np.float32)
    lnbw = np.asarray(inputs["ln_bias_w"], np.float32)
    lnbb = np.asarray(inputs["ln_bias_b"], np.float32)
    assert np.abs(lnpb).max() == 0.0 and np.abs(lnbb).max() == 0.0, \
        "kernel specialized for zero LN biases"
    Wq = np.asarray(inputs["Wq"], np.float32)
    Wk = np.asarray(inputs["Wk"], np.float32)
    Wv = np.asarray(inputs["Wv"], np.float32)
    Wb = np.asarray(inputs["Wb"], np.float32)
    Wg = np.asarray(inputs["Wg"], np.float32)
    bg = np.asarray(inputs["bg"], np.float32)
    Wo = np.asarray(inputs["Wo"], np.float32)
    bo = np.asarray(inputs["bo"], np.float32)

    pairT = np.ascontiguousarray(pair[0].transpose(1, 0, 2))   # X_raw[n, m, c]
    biasT = np.ascontiguousarray(bias[0].transpose(1, 0, 2))

    # permutation for einsum output partitions: P = d*4 + hq (per half)
    perm = np.empty(HD, np.int64)
    for half in range(2):
        for hq in range(4):
            for d_ in range(DH):
                perm[half * 128 + d_ * 4 + hq] = (4 * half + hq) * DH + d_
    wg_perm = (lnpw[:, None] * Wg)[:, perm]
    ones32 = np.full((D, 32), 1.0 / 128.0, np.float32)
    w_all = np.concatenate([
        (lnpw[:, None] * Wq) / math.sqrt(DH),
        (lnpw[:, None] * Wk) / math.sqrt(L),
        lnpw[:, None] * Wv,
        wg_perm,
        np.tile(lnbw[:, None] * Wb, (1, 4)),
        ones32,
    ], axis=1)
    wo_p = Wo[perm, :]
    wo2 = np.concatenate([wo_p[0:128, :], wo_p[128:256, :]], axis=1)
    bg_perm = bg[perm]

    base = {
        "w_all": np.ascontiguousarray(w_all.astype(F16)),
        "wo2": np.ascontiguousarray(wo2.astype(F16)),
        "bo": bo.reshape(D, 1).copy(),
        "bg2": np.ascontiguousarray(bg_perm.reshape(2, 128).T),
        "ident": np.eye(128, dtype=np.float32).astype(F16),
    }
    in_maps = []
    for c in range(N_CORES):
        m = dict(base)
        # pr[chan, r*2048 + g*256 + i] = X_raw[32c + 4g + r, i, chan]
        pr_rows = pairT[32 * c:32 * c + 32]            # [n_local, i, chan]
        pr_rgi = pr_rows.reshape(8, 4, L, D).transpose(1, 0, 2, 3)  # r,g,i,c
        m["pr"] = np.ascontiguousarray(
            pr_rgi.reshape(NTOK, D).T.astype(F16))
        # pc[chan, g*1024 + r*256 + i] = X_raw[i, 32c + 4g + r, chan]
        pc_cols = pairT[:, 32 * c:32 * c + 32]         # [i, k_local, chan]
        pc_gri = pc_cols.transpose(1, 0, 2).reshape(8, 4, L, D) \
            .transpose(0, 1, 2, 3)                     # (4g+r) major
        # k_local = 4g + r -> index [g, r]: k_local axis is (g*4 + ... wait
        pc_kli = pc_cols.transpose(1, 0, 2)            # [k_local, i, chan]
        pc_gr = pc_kli.reshape(8, 4, L, D)             # [g, r, i, c] k=4g+r
        m["pc"] = np.ascontiguousarray(
            pc_gr.reshape(NTOK, D).T.astype(F16))
        # br[chan, s*256 + j] = biasT[32c + s, j, chan]
        m["br"] = np.ascontiguousarray(
            biasT[32 * c:32 * c + 32].reshape(NTOK, D).T.astype(F16))
        in_maps.append(m)
    return in_maps


def _sharded_fn(nc):
    """Build (once) a cached jitted shard_map callable for the program."""
    if "fn" in _cache:
        return _cache["fn"]
    import jax
    import numpy as _np
    from jax.sharding import Mesh, PartitionSpec
    from jax.experimental.shard_map import shard_map
    from concourse import mybir
    from concourse import bass2jax as b2j

    b2j.install_neuronx_cc_hook()
    pid_name = (nc.partition_id_tensor.name
                if nc.partition_id_tensor is not None else None)
    in_names, out_names, out_shapes, out_dtypes = [], [], [], []
    for alloc in nc.m.functions[0].allocations:
        if not isinstance(alloc, mybir.MemoryLocationSet):
            continue
        name = alloc.memorylocations[0].name
        if alloc.kind == "ExternalInput":
            if name == pid_name:
                continue
            in_names.append(name)
        elif alloc.kind == "ExternalOutput":
            out_names.append(name)
            out_shapes.append(tuple(alloc.tensor_shape))
            out_dtypes.append(mybir.dt.np(alloc.dtype))
    n_params = len(in_names)
    n_outs = len(out_names)
    out_avals = [jax.core.ShapedArray(s, d)
                 for s, d in zip(out_shapes, out_dtypes)]
    all_names = in_names + out_names
    if pid_name is not None:
        all_names = all_names + [pid_name]

    def _body(*args):
        ops = list(args)
        if pid_name is not None:
            ops.append(b2j.partition_id_tensor())
        outs = b2j._bass_exec_p.bind(
            *ops,
            out_avals=tuple(out_avals),
            in_names=tuple(all_names),
            out_names=tuple(out_names),
            lowering_input_output_aliases=(),
            sim_require_finite=True,
            sim_require_nnan=True,
            nc=nc,
        )
        return tuple(outs)

    devices = jax.devices()[:N_CORES]
    mesh = Mesh(_np.asarray(devices), ("core",))
    in_specs = (PartitionSpec("core"),) * (n_params + n_outs)
    out_specs = (PartitionSpec("core"),) * n_outs
    donate = tuple(range(n_params, n_params + n_outs))
    fn = jax.jit(
        shard_map(_body, mesh=mesh, in_specs=in_specs, out_specs=out_specs,
                  check_rep=False),
        donate_argnums=donate, keep_unused=True)
    _cache["fn"] = (fn, in_names, out_names, out_shapes, out_dtypes)
    return _cache["fn"]


def kernel(**inputs):
    nc = _build()
    in_maps = _prep_inputs(inputs)
    fn, in_names, out_names, out_shapes, out_dtypes = _sharded_fn(nc)
    concat_in = [np.concatenate([in_maps[c][n] for c in range(N_CORES)],
                                axis=0)
                 for n in in_names]
    concat_zeros = [np.zeros((N_CORES * s[0], *s[1:]), d)
                    for s, d in zip(out_shapes, out_dtypes)]
    out_arrs = fn(*concat_in, *concat_zeros)
    oc_all = np.asarray(out_arrs[out_names.index("out")]) \
        .reshape(N_CORES, D, NTOK).astype(np.float32)
    out = np.empty((1, L, L, D), dtype=np.float32)
    for c in range(N_CORES):
        # col = g*1024 + kl*256 + i ; k_local = 4g + kl
        oc = oc_all[c].reshape(D, 8, 4, L)       # [D, g, kl, i]
        out[0, 32 * c:32 * c + 32] = \
            oc.transpose(1, 2, 3, 0).reshape(NS, L, D)
    return out


if __name__ == "__main__":
    _build()
    print("build ok")


# revision 34
# speedup vs baseline: 1.1261x; 1.1261x over previous
"""Trainium2 Bass kernel for BiasedAxialAttention (tied row attention), 8-core SPMD.

Math (reference, in the transposed frame X = LN(pairT), pairT[a,b,:] = pair[0,b,a,:]):
    q,k,v = X@Wq,Wk,Wv (scaled);  b = LN(biasT)@Wb
    g = sigmoid(X@Wg + bg)
    logits[i,j,h] = sum_{n,d} q[n,i,h,d] k[n,j,h,d] + b[i,j,h]
    attn = softmax_j(logits);  out[i,k,(h,d)] = sum_j attn[i,j,h] v[k,j,h,d]
    out = (g * out) @ Wo + bo;  final[k,i,:] = out[i,k,:]

Sharding (core c of 8): rows R_c = [32c, 32c+32) of X are both the tied
contraction rows (n) and the core's output columns (k). Partial logits are
AllReduduced in fp16 (the bias b is folded into the partials pre-AR, so no
AllGather is needed).

Host ships activations fp16, channel-major (pre-transposed):
    pr[chan, r*2048 + g*256 + i] = X_raw[32c + 4g + r, i, chan]
    pc[chan, g*1024 + r*256 + i] = X_raw[i, 32c + 4g + r, chan]
    br[chan, s*256 + j]          = biasT_raw[32c + s, j, chan]
LN runs on-device: per-token sums via ones-matmuls (replicated over 32-row
PSUM groups via tile_position), finishing on a [128,64] reshape, then the
(r | -m*r) row is partition-broadcast (GPSIMD) and applied as two
tensor_tensor passes. LN gamma and all static scales are folded into the
weights on the host; LN beta must be zero and the mask all-ones (asserted).
"""
import os
import sys

for _p in ("/opt/trn_rl_repo", "/root/.axon_site/_ro/trn_rl_repo"):
    if os.path.isdir(_p) and _p not in sys.path:
        sys.path.append(_p)

import math
import numpy as np

N_CORES = 8
L = 256
D = 128
H = 8
DH = 32
HD = H * DH          # 256
NS = L // N_CORES    # 32
NTOK = NS * L        # 8192
EPS = 1e-5

_cache = {}


def _build():
    if "nc" in _cache:
        return _cache["nc"]
    from contextlib import ExitStack

    import concourse.bacc as bacc
    import concourse.bass as cbass
    import concourse.tile as tile
    from concourse import mybir

    F32 = mybir.dt.float32
    F16 = mybir.dt.float16
    AF = mybir.ActivationFunctionType
    ALU = mybir.AluOpType

    nc = bacc.Bacc("TRN2", target_bir_lowering=False, debug=False,
                   num_devices=N_CORES)

    ei = dict(kind="ExternalInput")
    pr_d = nc.dram_tensor("pr", [D, NTOK], F16, **ei)
    pc_d = nc.dram_tensor("pc", [D, NTOK], F16, **ei)
    br_d = nc.dram_tensor("br", [D, NTOK], F16, **ei)
    # w_all: [D, 4*HD + 32 + 32]: wq|wk|wv|wg|wb_x4|ones32 (gamma, scales folded)
    WCOL = 4 * HD + 32 + 32
    wall_d = nc.dram_tensor("w_all", [D, WCOL], F16, **ei)
    wo2_d = nc.dram_tensor("wo2", [128, 2 * D], F16, **ei)
    bo_d = nc.dram_tensor("bo", [D, 1], F32, **ei)
    bg_d = nc.dram_tensor("bg2", [128, 2], F32, **ei)
    id_d = nc.dram_tensor("ident", [128, 128], F16, **ei)

    out_d = nc.dram_tensor("out", [D, NTOK], F16, kind="ExternalOutput")

    with tile.TileContext(nc) as tc, ExitStack() as ctx:
        singles = ctx.enter_context(tc.tile_pool(name="singles", bufs=1))
        small = ctx.enter_context(tc.tile_pool(name="small", bufs=1))
        p2 = ctx.enter_context(tc.tile_pool(name="p2", bufs=2))
        big = ctx.enter_context(tc.tile_pool(name="big", bufs=1))
        ps = ctx.enter_context(tc.tile_pool(name="ps", bufs=2, space="PSUM"))
        ps_st = ctx.enter_context(tc.tile_pool(name="ps_st", bufs=1,
                                               space="PSUM"))
        ps_t = ctx.enter_context(tc.tile_pool(name="ps_t", bufs=2,
                                              space="PSUM"))
        dram = ctx.enter_context(tc.tile_pool(name="dram", bufs=1,
                                              space="DRAM"))

        # ---------------- constants ----------------
        w_sb = singles.tile([128, WCOL], F16, tag="w_sb")
        nc.scalar.dma_start(w_sb[:], wall_d.ap())
        wq = w_sb[:, 0:HD]
        wk = w_sb[:, HD:2 * HD]
        wv = w_sb[:, 2 * HD:3 * HD]
        wg = w_sb[:, 3 * HD:4 * HD]
        wb = w_sb[:, 4 * HD:4 * HD + 32]
        ones32 = w_sb[:, 4 * HD + 32:4 * HD + 64]
        wo_sb = singles.tile([128, 2 * D], F16, tag="wo_sb")
        nc.scalar.dma_start(wo_sb[:], wo2_d.ap())
        bo_sb = singles.tile([128, 1], F32, tag="bo")
        nc.scalar.dma_start(bo_sb[:], bo_d.ap())
        bg_sb = singles.tile([128, 2], F32, tag="bg")
        nc.scalar.dma_start(bg_sb[:], bg_d.ap())
        ident = singles.tile([128, 128], F16, tag="ident")
        nc.scalar.dma_start(ident[:], id_d.ap())
        eps_t = singles.tile([128, 1], F32, tag="eps")
        nc.vector.memset(eps_t[:], EPS)

        # ---------------- input loads (4 chunks each) -----------------------
        xt_br = big.tile([128, NTOK], F16, tag="xt_br")
        xt_pr = big.tile([128, NTOK], F16, tag="xt_pr")
        xt_pc = big.tile([128, NTOK], F16, tag="xt_pc")
        for t4 in range(4):
            sl = slice(2048 * t4, 2048 * (t4 + 1))
            nc.sync.dma_start(xt_br[:, sl], br_d.ap()[:, sl])
        for t4 in range(4):
            sl = slice(2048 * t4, 2048 * (t4 + 1))
            nc.sync.dma_start(xt_pr[:, sl], pr_d.ap()[:, sl])
        for t4 in range(4):
            sl = slice(2048 * t4, 2048 * (t4 + 1))
            nc.scalar.dma_start(xt_pc[:, sl], pc_d.ap()[:, sl])

        # ---------------- DRAM collective tiles -----------------------------
        ar_in = [dram.tile([4, 2, 128, L], F16, tag=f"ar_in{i}",
                           name=f"ar_in{i}") for i in range(2)]
        ar_out = [dram.tile([4, 2, 128, L], F16, tag=f"ar_out{i}",
                            name=f"ar_out{i}", addr_space="Shared")
                  for i in range(2)]

        # =============== layernorm: stats + normalize ========================
        # stats: Sx(t) and Sq(t) (pre-scaled by 1/128) for all 8192 tokens.
        # ones32-matmul replicates each chunk's sums over a 32-row PSUM group;
        # chunk j -> rows [32*(j//4), +32), cols [512*(j%4), +512).
        def ln_stats_a(xt, nm):
            sq = big.tile([128, NTOK], F16, tag="ppq", name="sq")
            for blk in range(4):
                nc.vector.tensor_tensor(
                    out=sq[:, 2048 * blk:2048 * (blk + 1)],
                    in0=xt[:, 2048 * blk:2048 * (blk + 1)],
                    in1=xt[:, 2048 * blk:2048 * (blk + 1)], op=ALU.mult)
            m128 = small.tile([128, 64], F16, tag=f"m128_{nm}",
                              name=f"m128_{nm}")
            e128 = small.tile([128, 64], F16, tag=f"e128_{nm}",
                              name=f"e128_{nm}")
            for st in range(2):
                mps = ps_st.tile([128, 2048], F32, tag="stat",
                                 name=f"mps{st}")
                srcbuf = xt if st == 0 else sq
                for j in range(16):   # 512-token chunks
                    rg, slot = j // 4, j % 4
                    nc.tensor.matmul(
                        mps[32 * rg:32 * (rg + 1),
                            512 * slot:512 * (slot + 1)], ones32,
                        srcbuf[:, 512 * j:512 * (j + 1)],
                        start=True, stop=True, tile_position=(0, 32 * rg))
                # evac full replicated tile (engine partition access must
                # be contiguous); the DMA then picks rows {0,32,64,96} and
                # reshapes straight to [128, 64] (t = 64p + f), SBUF->SBUF.
                sx4 = small.tile([128, 2048], F16, tag="sx4",
                                 name=f"sx4_{nm}{st}")
                if st == 0:
                    nc.vector.tensor_copy(sx4[:], mps[:])
                else:
                    nc.scalar.activation(out=sx4[:], in_=mps[:], func=AF.Copy)
                st_d = dram.tile([4, 2048], F16, tag=f"st_d_{nm}{st}",
                                 name=f"st_d_{nm}{st}")
                nc.sync.dma_start(
                    st_d[:],
                    sx4[:].rearrange("(a b) f -> a b f", b=32)[:, 0, :])
                dst = m128 if st == 0 else e128
                nc.sync.dma_start(
                    dst[:], st_d[:].rearrange("a (b f) -> (a b) f", f=64))
            return m128, e128

        def ln_stats_b(me, nm):
            m128, e128 = me
            # finishing: var = E[x^2] - m^2 ; r = 1/sqrt(var+eps); nmr = -m*r
            var = small.tile([128, 64], F16, tag="var")
            nc.vector.scalar_tensor_tensor(
                out=var[:], in0=m128[:], scalar=-1.0, in1=m128[:],
                op0=ALU.mult, op1=ALU.mult)            # -m^2
            nc.vector.tensor_tensor(out=var[:], in0=e128[:], in1=var[:],
                                    op=ALU.add)        # E[x^2]-m^2
            std = small.tile([128, 64], F32, tag="std")
            nc.scalar.activation(out=std[:], in_=var[:], func=AF.Sqrt,
                                 bias=eps_t[:], scale=1.0)
            rec = small.tile([128, 64], F32, tag="rec")
            nc.vector.reciprocal(out=rec[:], in_=std[:])
            r16 = small.tile([128, 64], F16, tag="r16")
            nc.vector.tensor_copy(r16[:], rec[:])
            nmr = small.tile([128, 64], F16, tag="nmr")
            nc.vector.scalar_tensor_tensor(
                out=nmr[:], in0=m128[:], scalar=-1.0, in1=rec[:],
                op0=ALU.mult, op1=ALU.mult)
            # rows to DRAM (broadcast source); t = 64*p + f
            rm_d = dram.tile([2, 128, 64], F16, tag=f"rm_d_{nm}",
                             name=f"rm_d_{nm}")
            nc.sync.dma_start(rm_d[0], r16[:])
            nc.sync.dma_start(rm_d[1], nmr[:])
            return rm_d

        def ln_norm(xt, rm_d):
            # broadcast r / -m*r rows from DRAM (stride-0 partition source)
            # into a [128, NTOK] SBUF tile, applied as two TT passes.
            rm_bc = big.tile([128, NTOK], F16, tag="rm_bc", name="rm_bc")
            nc.scalar.dma_start(
                rm_bc[:],
                rm_d[0].rearrange("p f -> (p f)").partition_broadcast(128))
            for blk in range(4):
                sl = slice(2048 * blk, 2048 * (blk + 1))
                nc.vector.tensor_tensor(out=xt[:, sl], in0=xt[:, sl],
                                        in1=rm_bc[:, sl], op=ALU.mult)
            nc.scalar.dma_start(
                rm_bc[:],
                rm_d[1].rearrange("p f -> (p f)").partition_broadcast(128))
            for blk in range(4):
                sl = slice(2048 * blk, 2048 * (blk + 1))
                nc.vector.tensor_tensor(out=xt[:, sl], in0=xt[:, sl],
                                        in1=rm_bc[:, sl], op=ALU.add)

        me_br = ln_stats_a(xt_br, "br")
        me_pr = ln_stats_a(xt_pr, "pr")
        me_pc = ln_stats_a(xt_pc, "pc")
        rm_br = ln_stats_b(me_br, "br")
        rm_pr = ln_stats_b(me_pr, "pr")
        rm_pc = ln_stats_b(me_pc, "pc")
        ln_norm(xt_br, rm_br)
        ln_norm(xt_pr, rm_pr)
        ln_norm(xt_pc, rm_pc)

        # ====== b projection -> b_d DRAM [4 t4, 8 h, (8 s8, 256 j)] =========
        # b[h, s = 8*t4 + s8, j] lives at b_d[t4, h, (s8, j)]; the pre-AR
        # accumulate DMAs read it DRAM->DRAM with the CCE add.
        b_d = dram.tile([4, 8, 2048], F16, tag="b_d", name="b_d")
        for t4 in range(4):
            pb = ps_st.tile([32, 2048], F32, tag="stat", name="b_ps")
            for j in range(4):
                nc.tensor.matmul(pb[:, 512 * j:512 * (j + 1)], wb,
                                 xt_br[:, 2048 * t4 + 512 * j:
                                       2048 * t4 + 512 * (j + 1)],
                                 start=True, stop=True)
            bh = p2.tile([8, 2048], F16, tag="b_hd", name=f"bh{t4}")
            if t4 % 2 == 0:
                nc.vector.tensor_copy(bh[:], pb[0:8, :])
            else:
                nc.scalar.activation(out=bh[:], in_=pb[0:8, :], func=AF.Copy)
            nc.scalar.dma_start(b_d[t4], bh[:])

        # =============== q/k projection + pack + logits per half =============
        ppq = big.tile([128, NTOK], F16, tag="ppq")
        ppk = big.tile([128, NTOK], F16, tag="ppk")
        pkq_all = big.tile([128, 4, 8, L], F16, tag="xt_br", name="pkq_all")
        pkk_all = big.tile([128, 4, 8, L], F16, tag="pkk_all",
                           name="pkk_all")
        pkq = [pkq_all[:, i] for i in range(4)]
        pkk = [pkk_all[:, i] for i in range(4)]

        def evac(dst, src, k):
            if k % 3 in (0, 1):
                nc.scalar.activation(out=dst, in_=src, func=AF.Copy)
            else:
                nc.vector.tensor_copy(dst, src)

        def proj_half(w_, half, dst):
            for ch in range(16):
                pm = ps.tile([128, 512], F32, tag="mm")
                nc.tensor.matmul(pm[:], w_[:, 128 * half:128 * (half + 1)],
                                 dst[1][:, 512 * ch:512 * (ch + 1)],
                                 start=True, stop=True)
                evac(dst[0][:, 512 * ch:512 * (ch + 1)], pm[:], ch)

        ls_stage = [p2.tile([128, 4, 2, L], F16, tag="lss", name=f"lss{i}")
                    for i in range(2)]

        def qk_half(half):
            proj_half(wq, half, (ppq, xt_pr))
            proj_half(wk, half, (ppk, xt_pr))
            for hq in range(4):
                for r in range(4):
                    nc.sync.dma_start(
                        pkq[hq][32 * r:32 * (r + 1), :, :]
                        .rearrange("d g i -> d (g i)"),
                        ppq[32 * hq:32 * (hq + 1),
                            2048 * r:2048 * (r + 1)])
                    nc.scalar.dma_start(
                        pkk[hq][32 * r:32 * (r + 1), :, :]
                        .rearrange("d g i -> d (g i)"),
                        ppk[32 * hq:32 * (hq + 1),
                            2048 * r:2048 * (r + 1)])
            lss = ls_stage[half]
            for hq in range(4):
                pl = ps.tile([128, 2, L], F32, tag="mm", name="pl")
                for ih in range(2):
                    for g in range(8):
                        nc.tensor.matmul(
                            pl[:, ih, :],
                            pkq[hq][:, g, 128 * ih:128 * (ih + 1)],
                            pkk[hq][:, g, :], start=(g == 0), stop=(g == 7))
                evac(lss[:, hq, :, :].rearrange("p a b -> p (a b)"),
                     pl[:].rearrange("p a b -> p (a b)"), hq)
            return lss

        # fold b into the partial logits pre-AR: core c owns global rows
        # i = 32*(c%4) + s in half ih_c = c//4; DMA-accumulate b_t into
        # ar_in at a core-id-dependent DRAM offset (CCE add on the DMA).
        if not os.environ.get("KNOB_NO_BADD"):
            pid = nc.gpsimd.partition_id()
            ih_reg = pid // 4
            pc_reg = (pid % 4) * 32

        def run_half(half):
            lss = qk_half(half)
            nc.sync.dma_start(
                ar_in[half][:].rearrange("hq ih p j -> p hq ih j"), lss[:])
            if not os.environ.get("KNOB_NO_BADD"):
                for t4 in range(4):
                    nc.gpsimd.dma_start(
                        ar_in[half][:, cbass.ds(ih_reg, 1),
                                    cbass.ds(pc_reg + 8 * t4, 8), :],
                        b_d[t4, 4 * half:4 * (half + 1), :]
                        .rearrange("h (s j) -> h s j", s=8),
                        accum_op=ALU.add)
            if os.environ.get("KNOB_NO_AR"):
                nc.scalar.dma_start(ar_out[half][:], ar_in[half][:])
            else:
                nc.gpsimd.collective_compute(
                    "AllReduce", ALU.add,
                    replica_groups=[list(range(N_CORES))],
                    ins=[ar_in[half].opt()], outs=[ar_out[half].opt()],
                )

        run_half(0)
        run_half(1)

        # =============== v projection -> vkg[g] ==============================
        vkg_all = big.tile([128, 8, 2, 8, 4, DH], F16, tag="vkg_all",
                           name="vkg_all")
        vkg = [vkg_all[:, g] for g in range(8)]
        kv = 0
        for g in range(8):
            for r in range(4):
                for ih in range(2):
                    coff = 2048 * r + 256 * g + 128 * ih
                    pv = ps.tile([128, 256], F32, tag="mm", name="pv")
                    nc.tensor.matmul(pv[:], xt_pr[:, coff:coff + 128], wv,
                                     start=True, stop=True)
                    dst = vkg[g][:, ih, :, r, :]
                    src = pv[:].rearrange("p (h d) -> p h d", h=8)
                    evac(dst, src, kv)
                    kv += 1

        # =============== gate projection (sigmoid) ===========================
        gs = [big.tile([128, NTOK], F16, tag="ppq", name="gs0"),
              big.tile([128, NTOK], F16, tag="ppk", name="gs1")]
        for q in range(2):
            for ch in range(16):
                pg = ps.tile([128, 512], F32, tag="mm", name="pg")
                nc.tensor.matmul(pg[:], wg[:, 128 * q:128 * (q + 1)],
                                 xt_pc[:, 512 * ch:512 * (ch + 1)],
                                 start=True, stop=True)
                nc.scalar.activation(
                    out=gs[q][:, 512 * ch:512 * (ch + 1)], in_=pg[:],
                    func=AF.Sigmoid, bias=bg_sb[:, q:q + 1], scale=1.0)

        # =============== softmax per half -> at_t ============================
        at_t = big.tile([128, 2, 8, 2, 128], F16, tag="xt_br", name="at_t")
        atn = p2.tile([128, 4, 2, L], F16, tag="lss", name="atn")

        def softmax(half):
            lsb = p2.tile([128, 4, 2, L], F16, tag="lsb", bufs=1)
            nc.sync.dma_start(
                lsb[:], ar_out[half][:].rearrange("hq ih p j -> p hq ih j"))
            sm_s = small.tile([128, 8], F32, tag="sm_s")
            nmx = small.tile([128, 8], F32, tag="nmx")
            for hq in range(4):
                for ih in range(2):
                    cc = 2 * hq + ih
                    nc.vector.tensor_reduce(
                        out=nmx[:, cc:cc + 1], in_=lsb[:, hq, ih, :],
                        axis=mybir.AxisListType.X, op=ALU.max, negate=True)
            for hq in range(4):
                for ih in range(2):
                    cc = 2 * hq + ih
                    nc.scalar.activation(
                        out=atn[:, hq, ih, :], in_=lsb[:, hq, ih, :],
                        func=AF.Exp, bias=nmx[:, cc:cc + 1], scale=1.0,
                        accum_out=sm_s[:, cc:cc + 1])
            sm_r = small.tile([128, 8], F32, tag="sm_r")
            nc.vector.reciprocal(out=sm_r[:], in_=sm_s[:])
            for hq in range(4):
                h = 4 * half + hq
                for ih in range(2):
                    nc.vector.tensor_scalar(
                        out=atn[:, hq, ih, :], in0=atn[:, hq, ih, :],
                        scalar1=sm_r[:, 2 * hq + ih:2 * hq + ih + 1],
                        scalar2=None, op0=ALU.mult)
                pt = ps_t.tile([128, 2, 2, 128], F16, tag="pt")
                for ih in range(2):
                    for jh in range(2):
                        nc.tensor.transpose(
                            pt[:, ih, jh, :],
                            atn[:, hq, ih, 128 * jh:128 * (jh + 1)],
                            ident[:])
                evac(at_t[:, :, h, :, :].rearrange("p jh ih i -> p ih jh i"),
                     pt[:], hq)

        softmax(0)

        # ======= einsum phase A (heads 0-3 -> even h' -> okg q=0) ==========
        # runs inside the AR1 shadow; phase B (heads 4-7, q=1) follows
        # softmax(1), then the Wo projection consumes both.
        okg_all = big.tile([128, 8, 2, 4, L], F16, tag="xt_pr",
                           name="okg_all")

        def ein_half(g, hb):
            ein = p2.tile([128, 8, L], F16, tag="ein", name=f"ein{g}_{hb}")
            for hp in (2 * hb, 2 * hb + 1):
                hps = hp if hb == 0 else hp - 2
                po = ps.tile([128, 2, L], F32, tag="mm", name="po")
                for hh in range(2):
                    h = (4 * hb) + 2 * hps + hh if False else 2 * hp + hh
                    for jh in range(2):
                        nc.tensor.matmul(
                            po[:, hh, :],
                            vkg[g][:, jh, h, :, :]
                            .rearrange("p kl d -> p (kl d)"),
                            at_t[:, jh, h, :, :]
                            .rearrange("p ih i -> p (ih i)"),
                            start=(jh == 0), stop=(jh == 1))
                dst = ein[:].rearrange("p (k par) i -> p par k i", par=2)[
                    :, hp // 2, 2 * (hp % 2):2 * (hp % 2) + 2, :]
                evac(dst, po[:], hp + g)
            q = hb
            for kl in range(4):
                eng = (nc.sync, nc.scalar, nc.gpsimd, nc.gpsimd)[kl]
                eng.dma_start(
                    okg_all[:, g, q, kl, :],
                    ein[32 * kl:32 * (kl + 1), :, :]
                    .rearrange("d (hq q) i -> d q hq i", q=2)[:, q, :, :])
            nc.vector.tensor_tensor(
                out=okg_all[:, g, q, :, :].rearrange("p kl i -> p (kl i)"),
                in0=okg_all[:, g, q, :, :].rearrange("p kl i -> p (kl i)"),
                in1=gs[q][:, 1024 * g:1024 * (g + 1)], op=ALU.mult)

        def wo_block(g):
            ot = p2.tile([128, 1024], F16, tag="ot", name=f"ot{g}",
                         bufs=1)
            for cc in range(2):
                pw = ps.tile([128, 512], F32, tag="mm", name="pw")
                nc.tensor.matmul(pw[:], wo_sb[:, 0:D],
                                 okg_all[:, g, 0, 2 * cc:2 * (cc + 1), :]
                                 .rearrange("p kl i -> p (kl i)"),
                                 start=True, stop=False)
                nc.tensor.matmul(pw[:], wo_sb[:, D:2 * D],
                                 okg_all[:, g, 1, 2 * cc:2 * (cc + 1), :]
                                 .rearrange("p kl i -> p (kl i)"),
                                 start=False, stop=True)
                if (g + cc) % 2 == 0:
                    nc.scalar.activation(out=ot[:, 512 * cc:512 * (cc + 1)],
                                         in_=pw[:], func=AF.Identity,
                                         bias=bo_sb[:], scale=1.0)
                else:
                    nc.vector.tensor_scalar(
                        out=ot[:, 512 * cc:512 * (cc + 1)], in0=pw[:],
                        scalar1=bo_sb[:], scalar2=None, op0=ALU.add)
            eng = (nc.sync, nc.scalar, nc.gpsimd)[g % 3]
            eng.dma_start(out_d.ap()[:, 1024 * g:1024 * (g + 1)], ot[:])

        for g in range(8):
            ein_half(g, 0)
        softmax(1)
        for g in range(8):
            ein_half(g, 1)
        for g in range(8):
            wo_block(g)

    nc.compile()
    _cache["nc"] = nc
    return nc


def _prep_inputs(inputs):
    import ml_dtypes
    F16 = np.float16
    pair = np.asarray(inputs["pair"], dtype=np.float32)
    bias = np.asarray(inputs["bias"], dtype=np.float32)
    mask = np.asarray(inputs["mask"])
    assert bool(mask.all()), "kernel specialized for all-ones mask"
    lnpw = np.asarray(inputs["ln_pair_w"], np.float32)
    lnpb = np.asarray(inputs["ln_pair_b"], np.float32)
    lnbw = np.asarray(inputs["ln_bias_w"], np.float32)
    lnbb = np.asarray(inputs["ln_bias_b"], np.float32)
    assert np.abs(lnpb).max() == 0.0 and np.abs(lnbb).max() == 0.0, \
        "kernel specialized for zero LN biases"
    Wq = np.asarray(inputs["Wq"], np.float32)
    Wk = np.asarray(inputs["Wk"], np.float32)
    Wv = np.asarray(inputs["Wv"], np.float32)
    Wb = np.asarray(inputs["Wb"], np.float32)
    Wg = np.asarray(inputs["Wg"], np.float32)
    bg = np.asarray(inputs["bg"], np.float32)
    Wo = np.asarray(inputs["Wo"], np.float32)
    bo = np.asarray(inputs["bo"], np.float32)

    pairT = np.ascontiguousarray(pair[0].transpose(1, 0, 2))   # X_raw[n, m, c]
    biasT = np.ascontiguousarray(bias[0].transpose(1, 0, 2))

    # permutation for einsum output partitions: P = d*4 + hq (per half)
    perm = np.empty(HD, np.int64)
    for half in range(2):
        for hq in range(4):
            for d_ in range(DH):
                perm[half * 128 + d_ * 4 + hq] = (4 * half + hq) * DH + d_
    wg_perm = (lnpw[:, None] * Wg)[:, perm]
    ones32 = np.full((D, 32), 1.0 / 128.0, np.float32)
    w_all = np.concatenate([
        (lnpw[:, None] * Wq) / math.sqrt(DH),
        (lnpw[:, None] * Wk) / math.sqrt(L),
        lnpw[:, None] * Wv,
        wg_perm,
        np.tile(lnbw[:, None] * Wb, (1, 4)),
        ones32,
    ], axis=1)
    wo_p = Wo[perm, :]
    wo2 = np.concatenate([wo_p[0:128, :], wo_p[128:256, :]], axis=1)
    bg_perm = bg[perm]

    base = {
        "w_all": np.ascontiguousarray(w_all.astype(F16)),
        "wo2": np.ascontiguousarray(wo2.astype(F16)),
        "bo": bo.reshape(D, 1).copy(),
        "bg2": np.ascontiguousarray(bg_perm.reshape(2, 128).T),
        "ident": np.eye(128, dtype=np.float32).astype(F16),
    }
    in_maps = []
    for c in range(N_CORES):
        m = dict(base)
        # pr[chan, r*2048 + g*256 + i] = X_raw[32c + 4g + r, i, chan]
        pr_rows = pairT[32 * c:32 * c + 32]            # [n_local, i, chan]
        pr_rgi = pr_rows.reshape(8, 4, L, D).transpose(1, 0, 2, 3)  # r,g,i,c
        m["pr"] = np.ascontiguousarray(
            pr_rgi.reshape(NTOK, D).T.astype(F16))
        # pc[chan, g*1024 + r*256 + i] = X_raw[i, 32c + 4g + r, chan]
        pc_cols = pairT[:, 32 * c:32 * c + 32]         # [i, k_local, chan]
        pc_gri = pc_cols.transpose(1, 0, 2).reshape(8, 4, L, D) \
            .transpose(0, 1, 2, 3)                     # (4g+r) major
        # k_local = 4g + r -> index [g, r]: k_local axis is (g*4 + ... wait
        pc_kli = pc_cols.transpose(1, 0, 2)            # [k_local, i, chan]
        pc_gr = pc_kli.reshape(8, 4, L, D)             # [g, r, i, c] k=4g+r
        m["pc"] = np.ascontiguousarray(
            pc_gr.reshape(NTOK, D).T.astype(F16))
        # br[chan, s*256 + j] = biasT[32c + s, j, chan]
        m["br"] = np.ascontiguousarray(
            biasT[32 * c:32 * c + 32].reshape(NTOK, D).T.astype(F16))
        in_maps.append(m)
    return in_maps


def _sharded_fn(nc):
    """Build (once) a cached jitted shard_map callable for the program."""
    if "fn" in _cache:
        return _cache["fn"]
    import jax
    import numpy as _np
    from jax.sharding import Mesh, PartitionSpec
    from jax.experimental.shard_map import shard_map
    from concourse import mybir
    from concourse import bass2jax as b2j

    b2j.install_neuronx_cc_hook()
    pid_name = (nc.partition_id_tensor.name
                if nc.partition_id_tensor is not None else None)
    in_names, out_names, out_shapes, out_dtypes = [], [], [], []
    for alloc in nc.m.functions[0].allocations:
        if not isinstance(alloc, mybir.MemoryLocationSet):
            continue
        name = alloc.memorylocations[0].name
        if alloc.kind == "ExternalInput":
            if name == pid_name:
                continue
            in_names.append(name)
        elif alloc.kind == "ExternalOutput":
            out_names.append(name)
            out_shapes.append(tuple(alloc.tensor_shape))
            out_dtypes.append(mybir.dt.np(alloc.dtype))
    n_params = len(in_names)
    n_outs = len(out_names)
    out_avals = [jax.core.ShapedArray(s, d)
                 for s, d in zip(out_shapes, out_dtypes)]
    all_names = in_names + out_names
    if pid_name is not None:
        all_names = all_names + [pid_name]

    def _body(*args):
        ops = list(args)
        if pid_name is not None:
            ops.append(b2j.partition_id_tensor())
        outs = b2j._bass_exec_p.bind(
            *ops,
            out_avals=tuple(out_avals),
            in_names=tuple(all_names),
            out_names=tuple(out_names),
            lowering_input_output_aliases=(),
            sim_require_finite=True,
            sim_require_nnan=True,
            nc=nc,
        )
        return tuple(outs)

    devices = jax.devices()[:N_CORES]
    mesh = Mesh(_np.asarray(devices), ("core",))
    in_specs = (PartitionSpec("core"),) * (n_params + n_outs)
    out_specs = (PartitionSpec("core"),) * n_outs
    donate = tuple(range(n_params, n_params + n_outs))
    fn = jax.jit(
        shard_map(_body, mesh=mesh, in_specs=in_specs, out_specs=out_specs,
                  check_rep=False),
        donate_argnums=donate, keep_unused=True)
    _cache["fn"] = (fn, in_names, out_names, out_shapes, out_dtypes)
    return _cache["fn"]


def kernel(**inputs):
    nc = _build()
    in_maps = _prep_inputs(inputs)
    fn, in_names, out_names, out_shapes, out_dtypes = _sharded_fn(nc)
    concat_in = [np.concatenate([in_maps[c][n] for c in range(N_CORES)],
                                axis=0)
                 for n in in_names]
    concat_zeros = [np.zeros((N_CORES * s[0], *s[1:]), d)
                    for s, d in zip(out_shapes, out_dtypes)]
    out_arrs = fn(*concat_in, *concat_zeros)
    oc_all = np.asarray(out_arrs[out_names.index("out")]) \
        .reshape(N_CORES, D, NTOK).astype(np.float32)
    out = np.empty((1, L, L, D), dtype=np.float32)
    for c in range(N_CORES):
        # col = g*1024 + kl*256 + i ; k_local = 4g + kl
        oc = oc_all[c].reshape(D, 8, 4, L)       # [D, g, kl, i]
        out[0, 32 * c:32 * c + 32] = \
            oc.transpose(1, 2, 3, 0).reshape(NS, L, D)
    return out


if __name__ == "__main__":
    _build()
    print("build ok")


# revision 36
# speedup vs baseline: 1.1671x; 1.0364x over previous
"""Trainium2 Bass kernel for BiasedAxialAttention (tied row attention), 8-core SPMD.

Math (reference, in the transposed frame X = LN(pairT), pairT[a,b,:] = pair[0,b,a,:]):
    q,k,v = X@Wq,Wk,Wv (scaled);  b = LN(biasT)@Wb
    g = sigmoid(X@Wg + bg)
    logits[i,j,h] = sum_{n,d} q[n,i,h,d] k[n,j,h,d] + b[i,j,h]
    attn = softmax_j(logits);  out[i,k,(h,d)] = sum_j attn[i,j,h] v[k,j,h,d]
    out = (g * out) @ Wo + bo;  final[k,i,:] = out[i,k,:]

Sharding (core c of 8): rows R_c = [32c, 32c+32) of X are both the tied
contraction rows (n) and the core's output columns (k). Partial logits are
AllReduduced in fp16 (the bias b is folded into the partials pre-AR, so no
AllGather is needed).

Host ships activations fp16, channel-major (pre-transposed):
    pr[chan, r*2048 + g*256 + i] = X_raw[32c + 4g + r, i, chan]
    pc[chan, g*1024 + r*256 + i] = X_raw[i, 32c + 4g + r, chan]
    br[chan, s*256 + j]          = biasT_raw[32c + s, j, chan]
LN runs on-device: per-token sums via ones-matmuls (replicated over 32-row
PSUM groups via tile_position), finishing on a [128,64] reshape, then the
(r | -m*r) row is partition-broadcast (GPSIMD) and applied as two
tensor_tensor passes. LN gamma and all static scales are folded into the
weights on the host; LN beta must be zero and the mask all-ones (asserted).
"""
import os
import sys

for _p in ("/opt/trn_rl_repo", "/root/.axon_site/_ro/trn_rl_repo"):
    if os.path.isdir(_p) and _p not in sys.path:
        sys.path.append(_p)

import math
import numpy as np

N_CORES = 8
L = 256
D = 128
H = 8
DH = 32
HD = H * DH          # 256
NS = L // N_CORES    # 32
NTOK = NS * L        # 8192
EPS = 1e-5

_cache = {}


def _build():
    if "nc" in _cache:
        return _cache["nc"]
    from contextlib import ExitStack

    import concourse.bacc as bacc
    import concourse.bass as cbass
    import concourse.tile as tile
    from concourse import mybir

    F32 = mybir.dt.float32
    F16 = mybir.dt.float16
    AF = mybir.ActivationFunctionType
    ALU = mybir.AluOpType

    nc = bacc.Bacc("TRN2", target_bir_lowering=False, debug=False,
                   num_devices=N_CORES)

    ei = dict(kind="ExternalInput")
    pr_d = nc.dram_tensor("pr", [D, NTOK], F16, **ei)
    pc_d = nc.dram_tensor("pc", [D, NTOK], F16, **ei)
    br_d = nc.dram_tensor("br", [D, NTOK], F16, **ei)
    # w_all: [D, 4*HD + 32 + 32]: wq|wk|wv|wg|wb_x4|ones32 (gamma, scales folded)
    WCOL = 4 * HD + 32 + 32
    wall_d = nc.dram_tensor("w_all", [D, WCOL], F16, **ei)
    wo2_d = nc.dram_tensor("wo2", [128, 2 * D], F16, **ei)
    bo_d = nc.dram_tensor("bo", [D, 1], F32, **ei)
    bg_d = nc.dram_tensor("bg2", [128, 2], F32, **ei)
    id_d = nc.dram_tensor("ident", [128, 128], F16, **ei)

    out_d = nc.dram_tensor("out", [D, NTOK], F16, kind="ExternalOutput")

    with tile.TileContext(nc) as tc, ExitStack() as ctx:
        singles = ctx.enter_context(tc.tile_pool(name="singles", bufs=1))
        small = ctx.enter_context(tc.tile_pool(name="small", bufs=1))
        p2 = ctx.enter_context(tc.tile_pool(name="p2", bufs=2))
        big = ctx.enter_context(tc.tile_pool(name="big", bufs=1))
        ps = ctx.enter_context(tc.tile_pool(name="ps", bufs=2, space="PSUM"))
        ps_st = ctx.enter_context(tc.tile_pool(name="ps_st", bufs=1,
                                               space="PSUM"))
        ps_t = ctx.enter_context(tc.tile_pool(name="ps_t", bufs=2,
                                              space="PSUM"))
        dram = ctx.enter_context(tc.tile_pool(name="dram", bufs=1,
                                              space="DRAM"))

        # ---------------- constants ----------------
        w_sb = singles.tile([128, WCOL], F16, tag="w_sb")
        nc.scalar.dma_start(w_sb[:], wall_d.ap())
        wq = w_sb[:, 0:HD]
        wk = w_sb[:, HD:2 * HD]
        wv = w_sb[:, 2 * HD:3 * HD]
        wg = w_sb[:, 3 * HD:4 * HD]
        wb = w_sb[:, 4 * HD:4 * HD + 32]
        ones32 = w_sb[:, 4 * HD + 32:4 * HD + 64]
        wo_sb = singles.tile([128, 2 * D], F16, tag="wo_sb")
        nc.scalar.dma_start(wo_sb[:], wo2_d.ap())
        bo_sb = singles.tile([128, 1], F32, tag="bo")
        nc.scalar.dma_start(bo_sb[:], bo_d.ap())
        bg_sb = singles.tile([128, 2], F32, tag="bg")
        nc.scalar.dma_start(bg_sb[:], bg_d.ap())
        ident = singles.tile([128, 128], F16, tag="ident")
        nc.scalar.dma_start(ident[:], id_d.ap())
        eps_t = singles.tile([128, 1], F32, tag="eps")
        nc.vector.memset(eps_t[:], EPS)

        # ---------------- input loads (4 chunks each) -----------------------
        xt_br = big.tile([128, NTOK], F16, tag="xt_br")
        xt_pr = big.tile([128, NTOK], F16, tag="xt_pr")
        xt_pc = big.tile([128, NTOK], F16, tag="xt_pc")
        for t4 in range(4):
            sl = slice(2048 * t4, 2048 * (t4 + 1))
            nc.sync.dma_start(xt_br[:, sl], br_d.ap()[:, sl])
        for t4 in range(4):
            sl = slice(2048 * t4, 2048 * (t4 + 1))
            nc.sync.dma_start(xt_pr[:, sl], pr_d.ap()[:, sl])
        for t4 in range(4):
            sl = slice(2048 * t4, 2048 * (t4 + 1))
            nc.scalar.dma_start(xt_pc[:, sl], pc_d.ap()[:, sl])

        # ---------------- DRAM collective tiles -----------------------------
        ar_in = [dram.tile([4, 2, 128, L], F16, tag=f"ar_in{i}",
                           name=f"ar_in{i}") for i in range(2)]
        ar_out = [dram.tile([4, 2, 128, L], F16, tag=f"ar_out{i}",
                            name=f"ar_out{i}", addr_space="Shared")
                  for i in range(2)]

        # =============== layernorm: stats + normalize ========================
        # stats: Sx(t) and Sq(t) (pre-scaled by 1/128) for all 8192 tokens.
        # ones32-matmul replicates each chunk's sums over a 32-row PSUM group;
        # chunk j -> rows [32*(j//4), +32), cols [512*(j%4), +512).
        def ln_stats_a(xt, nm):
            sq = big.tile([128, NTOK], F16, tag="ppq", name="sq")
            for blk in range(4):
                nc.vector.tensor_tensor(
                    out=sq[:, 2048 * blk:2048 * (blk + 1)],
                    in0=xt[:, 2048 * blk:2048 * (blk + 1)],
                    in1=xt[:, 2048 * blk:2048 * (blk + 1)], op=ALU.mult)
            m128 = small.tile([128, 64], F16, tag=f"m128_{nm}",
                              name=f"m128_{nm}")
            e128 = small.tile([128, 64], F16, tag=f"e128_{nm}",
                              name=f"e128_{nm}")
            for st in range(2):
                mps = ps_st.tile([128, 2048], F32, tag="stat",
                                 name=f"mps{st}")
                srcbuf = xt if st == 0 else sq
                for j in range(16):   # 512-token chunks
                    rg, slot = j // 4, j % 4
                    nc.tensor.matmul(
                        mps[32 * rg:32 * (rg + 1),
                            512 * slot:512 * (slot + 1)], ones32,
                        srcbuf[:, 512 * j:512 * (j + 1)],
                        start=True, stop=True, tile_position=(0, 32 * rg))
                # evac full replicated tile (engine partition access must
                # be contiguous); the DMA then picks rows {0,32,64,96} and
                # reshapes straight to [128, 64] (t = 64p + f), SBUF->SBUF.
                sx4 = small.tile([128, 2048], F16, tag="sx4",
                                 name=f"sx4_{nm}{st}")
                if st == 0:
                    nc.vector.tensor_copy(sx4[:], mps[:])
                else:
                    nc.scalar.activation(out=sx4[:], in_=mps[:], func=AF.Copy)
                st_d = dram.tile([4, 2048], F16, tag=f"st_d_{nm}{st}",
                                 name=f"st_d_{nm}{st}")
                nc.sync.dma_start(
                    st_d[:],
                    sx4[:].rearrange("(a b) f -> a b f", b=32)[:, 0, :])
                dst = m128 if st == 0 else e128
                nc.sync.dma_start(
                    dst[:], st_d[:].rearrange("a (b f) -> (a b) f", f=64))
            return m128, e128

        def ln_stats_b(me, nm):
            m128, e128 = me
            # finishing: var = E[x^2] - m^2 ; r = 1/sqrt(var+eps); nmr = -m*r
            var = small.tile([128, 64], F16, tag="var")
            nc.vector.scalar_tensor_tensor(
                out=var[:], in0=m128[:], scalar=-1.0, in1=m128[:],
                op0=ALU.mult, op1=ALU.mult)            # -m^2
            nc.vector.tensor_tensor(out=var[:], in0=e128[:], in1=var[:],
                                    op=ALU.add)        # E[x^2]-m^2
            std = small.tile([128, 64], F32, tag="std")
            nc.scalar.activation(out=std[:], in_=var[:], func=AF.Sqrt,
                                 bias=eps_t[:], scale=1.0)
            rec = small.tile([128, 64], F32, tag="rec")
            nc.vector.reciprocal(out=rec[:], in_=std[:])
            r16 = small.tile([128, 64], F16, tag="r16")
            nc.vector.tensor_copy(r16[:], rec[:])
            nmr = small.tile([128, 64], F16, tag="nmr")
            nc.vector.scalar_tensor_tensor(
                out=nmr[:], in0=m128[:], scalar=-1.0, in1=rec[:],
                op0=ALU.mult, op1=ALU.mult)
            # rows to DRAM (broadcast source); t = 64*p + f
            rm_d = dram.tile([2, 128, 64], F16, tag=f"rm_d_{nm}",
                             name=f"rm_d_{nm}")
            nc.sync.dma_start(rm_d[0], r16[:])
            nc.sync.dma_start(rm_d[1], nmr[:])
            return rm_d

        def ln_norm(xt, rm_d):
            # broadcast r / -m*r rows from DRAM (stride-0 partition source)
            # into [128, NTOK] SBUF tiles (chunked, on separate queues so the
            # TT passes start early), applied as two TT passes.
            for st, op in ((0, ALU.mult), (1, ALU.add)):
                for hh in range(2):
                    bc = p2.tile([128, 4096], F16, tag="rmb",
                                 name=f"rmb{st}{hh}")
                    eng = nc.scalar if st == 0 else nc.gpsimd
                    eng.dma_start(
                        bc[:],
                        rm_d[st].rearrange("p f -> (p f)")
                        [4096 * hh:4096 * (hh + 1)].partition_broadcast(128))
                    for b2 in range(2):
                        sl = slice(4096 * hh + 2048 * b2,
                                   4096 * hh + 2048 * (b2 + 1))
                        nc.vector.tensor_tensor(
                            out=xt[:, sl], in0=xt[:, sl],
                            in1=bc[:, 2048 * b2:2048 * (b2 + 1)], op=op)

        me_br = ln_stats_a(xt_br, "br")
        me_pr = ln_stats_a(xt_pr, "pr")
        me_pc = ln_stats_a(xt_pc, "pc")
        rm_br = ln_stats_b(me_br, "br")
        rm_pr = ln_stats_b(me_pr, "pr")
        rm_pc = ln_stats_b(me_pc, "pc")
        ln_norm(xt_br, rm_br)
        ln_norm(xt_pr, rm_pr)
        ln_norm(xt_pc, rm_pc)

        # ====== b projection -> b_d DRAM [4 t4, 8 h, (8 s8, 256 j)] =========
        # b[h, s = 8*t4 + s8, j] lives at b_d[t4, h, (s8, j)]; the pre-AR
        # accumulate DMAs read it DRAM->DRAM with the CCE add.
        b_d = dram.tile([4, 8, 2048], F16, tag="b_d", name="b_d")
        for t4 in range(4):
            pb = ps_st.tile([32, 2048], F32, tag="stat", name="b_ps")
            for j in range(4):
                nc.tensor.matmul(pb[:, 512 * j:512 * (j + 1)], wb,
                                 xt_br[:, 2048 * t4 + 512 * j:
                                       2048 * t4 + 512 * (j + 1)],
                                 start=True, stop=True)
            bh = p2.tile([8, 2048], F16, tag="b_hd", name=f"bh{t4}")
            if t4 % 2 == 0:
                nc.vector.tensor_copy(bh[:], pb[0:8, :])
            else:
                nc.scalar.activation(out=bh[:], in_=pb[0:8, :], func=AF.Copy)
            nc.scalar.dma_start(b_d[t4], bh[:])

        # =============== q/k projection + pack + logits per half =============
        ppq = big.tile([128, NTOK], F16, tag="ppq")
        ppk = big.tile([128, NTOK], F16, tag="ppk")
        pkq_all = big.tile([128, 4, 8, L], F16, tag="xt_br", name="pkq_all")
        pkk_all = big.tile([128, 4, 8, L], F16, tag="pkk_all",
                           name="pkk_all")
        pkq = [pkq_all[:, i] for i in range(4)]
        pkk = [pkk_all[:, i] for i in range(4)]

        def evac(dst, src, k):
            if k % 3 in (0, 1):
                nc.scalar.activation(out=dst, in_=src, func=AF.Copy)
            else:
                nc.vector.tensor_copy(dst, src)

        def proj_half(w_, half, dst):
            for ch in range(16):
                pm = ps.tile([128, 512], F32, tag="mm")
                nc.tensor.matmul(pm[:], w_[:, 128 * half:128 * (half + 1)],
                                 dst[1][:, 512 * ch:512 * (ch + 1)],
                                 start=True, stop=True)
                evac(dst[0][:, 512 * ch:512 * (ch + 1)], pm[:], ch)

        ls_stage = [p2.tile([128, 4, 2, L], F16, tag="lss", name=f"lss{i}")
                    for i in range(2)]

        def qk_half(half):
            proj_half(wq, half, (ppq, xt_pr))
            proj_half(wk, half, (ppk, xt_pr))
            for hq in range(4):
                for r in range(4):
                    nc.sync.dma_start(
                        pkq[hq][32 * r:32 * (r + 1), :, :]
                        .rearrange("d g i -> d (g i)"),
                        ppq[32 * hq:32 * (hq + 1),
                            2048 * r:2048 * (r + 1)])
                    nc.scalar.dma_start(
                        pkk[hq][32 * r:32 * (r + 1), :, :]
                        .rearrange("d g i -> d (g i)"),
                        ppk[32 * hq:32 * (hq + 1),
                            2048 * r:2048 * (r + 1)])
            lss = ls_stage[half]
            for hq in range(4):
                pl = ps.tile([128, 2, L], F32, tag="mm", name="pl")
                for ih in range(2):
                    for g in range(8):
                        nc.tensor.matmul(
                            pl[:, ih, :],
                            pkq[hq][:, g, 128 * ih:128 * (ih + 1)],
                            pkk[hq][:, g, :], start=(g == 0), stop=(g == 7))
                evac(lss[:, hq, :, :].rearrange("p a b -> p (a b)"),
                     pl[:].rearrange("p a b -> p (a b)"), hq)
            return lss

        # fold b into the partial logits pre-AR: core c owns global rows
        # i = 32*(c%4) + s in half ih_c = c//4; DMA-accumulate b_t into
        # ar_in at a core-id-dependent DRAM offset (CCE add on the DMA).
        if not os.environ.get("KNOB_NO_BADD"):
            pid = nc.gpsimd.partition_id()
            ih_reg = pid // 4
            pc_reg = (pid % 4) * 32

        def run_half(half):
            lss = qk_half(half)
            nc.sync.dma_start(
                ar_in[half][:].rearrange("hq ih p j -> p hq ih j"), lss[:])
            if not os.environ.get("KNOB_NO_BADD"):
                for t4 in range(4):
                    nc.gpsimd.dma_start(
                        ar_in[half][:, cbass.ds(ih_reg, 1),
                                    cbass.ds(pc_reg + 8 * t4, 8), :],
                        b_d[t4, 4 * half:4 * (half + 1), :]
                        .rearrange("h (s j) -> h s j", s=8),
                        accum_op=ALU.add)
            if os.environ.get("KNOB_NO_AR"):
                nc.scalar.dma_start(ar_out[half][:], ar_in[half][:])
            else:
                nc.gpsimd.collective_compute(
                    "AllReduce", ALU.add,
                    replica_groups=[list(range(N_CORES))],
                    ins=[ar_in[half].opt()], outs=[ar_out[half].opt()],
                )

        run_half(0)
        run_half(1)

        # =============== v projection -> vkg[g] ==============================
        vkg_all = big.tile([128, 8, 2, 8, 4, DH], F16, tag="vkg_all",
                           name="vkg_all")
        vkg = [vkg_all[:, g] for g in range(8)]
        kv = 0
        for g in range(8):
            for r in range(4):
                for ih in range(2):
                    coff = 2048 * r + 256 * g + 128 * ih
                    pv = ps.tile([128, 256], F32, tag="mm", name="pv")
                    nc.tensor.matmul(pv[:], xt_pr[:, coff:coff + 128], wv,
                                     start=True, stop=True)
                    dst = vkg[g][:, ih, :, r, :]
                    src = pv[:].rearrange("p (h d) -> p h d", h=8)
                    evac(dst, src, kv)
                    kv += 1

        # =============== gate projection (sigmoid) ===========================
        gs = [big.tile([128, NTOK], F16, tag="ppq", name="gs0"),
              big.tile([128, NTOK], F16, tag="ppk", name="gs1")]
        for q in range(2):
            for ch in range(16):
                pg = ps.tile([128, 512], F32, tag="mm", name="pg")
                nc.tensor.matmul(pg[:], wg[:, 128 * q:128 * (q + 1)],
                                 xt_pc[:, 512 * ch:512 * (ch + 1)],
                                 start=True, stop=True)
                nc.scalar.activation(
                    out=gs[q][:, 512 * ch:512 * (ch + 1)], in_=pg[:],
                    func=AF.Sigmoid, bias=bg_sb[:, q:q + 1], scale=1.0)

        # =============== softmax per half -> at_t ============================
        at_t = big.tile([128, 2, 8, 2, 128], F16, tag="xt_br", name="at_t")
        atn = p2.tile([128, 4, 2, L], F16, tag="lss", name="atn")

        def softmax(half):
            lsb = p2.tile([128, 4, 2, L], F16, tag="lsb", bufs=1)
            nc.sync.dma_start(
                lsb[:], ar_out[half][:].rearrange("hq ih p j -> p hq ih j"))
            sm_s = small.tile([128, 8], F32, tag="sm_s")
            nmx = small.tile([128, 8], F32, tag="nmx")
            for hq in range(4):
                for ih in range(2):
                    cc = 2 * hq + ih
                    nc.vector.tensor_reduce(
                        out=nmx[:, cc:cc + 1], in_=lsb[:, hq, ih, :],
                        axis=mybir.AxisListType.X, op=ALU.max, negate=True)
            for hq in range(4):
                for ih in range(2):
                    cc = 2 * hq + ih
                    nc.scalar.activation(
                        out=atn[:, hq, ih, :], in_=lsb[:, hq, ih, :],
                        func=AF.Exp, bias=nmx[:, cc:cc + 1], scale=1.0,
                        accum_out=sm_s[:, cc:cc + 1])
            sm_r = small.tile([128, 8], F32, tag="sm_r")
            nc.vector.reciprocal(out=sm_r[:], in_=sm_s[:])
            for hq in range(4):
                h = 4 * half + hq
                for ih in range(2):
                    nc.vector.tensor_scalar(
                        out=atn[:, hq, ih, :], in0=atn[:, hq, ih, :],
                        scalar1=sm_r[:, 2 * hq + ih:2 * hq + ih + 1],
                        scalar2=None, op0=ALU.mult)
                pt = ps_t.tile([128, 2, 2, 128], F16, tag="pt")
                for ih in range(2):
                    for jh in range(2):
                        nc.tensor.transpose(
                            pt[:, ih, jh, :],
                            atn[:, hq, ih, 128 * jh:128 * (jh + 1)],
                            ident[:])
                evac(at_t[:, :, h, :, :].rearrange("p jh ih i -> p ih jh i"),
                     pt[:], hq)

        softmax(0)

        # ======= einsum phase A (heads 0-3 -> even h' -> okg q=0) ==========
        # runs inside the AR1 shadow; phase B (heads 4-7, q=1) follows
        # softmax(1), then the Wo projection consumes both.
        okg_all = big.tile([128, 8, 2, 4, L], F16, tag="xt_pr",
                           name="okg_all")

        def ein_half(g, hb):
            ein = p2.tile([128, 8, L], F16, tag="ein", name=f"ein{g}_{hb}")
            for hp in (2 * hb, 2 * hb + 1):
                hps = hp if hb == 0 else hp - 2
                po = ps.tile([128, 2, L], F32, tag="mm", name="po")
                for hh in range(2):
                    h = (4 * hb) + 2 * hps + hh if False else 2 * hp + hh
                    for jh in range(2):
                        nc.tensor.matmul(
                            po[:, hh, :],
                            vkg[g][:, jh, h, :, :]
                            .rearrange("p kl d -> p (kl d)"),
                            at_t[:, jh, h, :, :]
                            .rearrange("p ih i -> p (ih i)"),
                            start=(jh == 0), stop=(jh == 1))
                dst = ein[:].rearrange("p (k par) i -> p par k i", par=2)[
                    :, hp // 2, 2 * (hp % 2):2 * (hp % 2) + 2, :]
                evac(dst, po[:], hp + g)
            q = hb
            for kl in range(4):
                eng = (nc.sync, nc.scalar, nc.gpsimd, nc.gpsimd)[kl]
                eng.dma_start(
                    okg_all[:, g, q, kl, :],
                    ein[32 * kl:32 * (kl + 1), :, :]
                    .rearrange("d (hq q) i -> d q hq i", q=2)[:, q, :, :])
            nc.vector.tensor_tensor(
                out=okg_all[:, g, q, :, :].rearrange("p kl i -> p (kl i)"),
                in0=okg_all[:, g, q, :, :].rearrange("p kl i -> p (kl i)"),
                in1=gs[q][:, 1024 * g:1024 * (g + 1)], op=ALU.mult)

        def wo_block(g):
            ot = p2.tile([128, 1024], F16, tag="ot", name=f"ot{g}",
                         bufs=1)
            for cc in range(2):
                pw = ps.tile([128, 512], F32, tag="mm", name="pw")
                nc.tensor.matmul(pw[:], wo_sb[:, 0:D],
                                 okg_all[:, g, 0, 2 * cc:2 * (cc + 1), :]
                                 .rearrange("p kl i -> p (kl i)"),
                                 start=True, stop=False)
                nc.tensor.matmul(pw[:], wo_sb[:, D:2 * D],
                                 okg_all[:, g, 1, 2 * cc:2 * (cc + 1), :]
                                 .rearrange("p kl i -> p (kl i)"),
                                 start=False, stop=True)
                if (g + cc) % 2 == 0:
                    nc.scalar.activation(out=ot[:, 512 * cc:512 * (cc + 1)],
                                         in_=pw[:], func=AF.Identity,
                                         bias=bo_sb[:], scale=1.0)
                else:
                    nc.vector.tensor_scalar(
                        out=ot[:, 512 * cc:512 * (cc + 1)], in0=pw[:],
                        scalar1=bo_sb[:], scalar2=None, op0=ALU.add)
            eng = (nc.sync, nc.scalar, nc.gpsimd)[g % 3]
            eng.dma_start(out_d.ap()[:, 1024 * g:1024 * (g + 1)], ot[:])

        for g in range(8):
            ein_half(g, 0)
        softmax(1)
        for g in range(8):
            ein_half(g, 1)
        for g in range(8):
            wo_block(g)

    nc.compile()
    _cache["nc"] = nc
    return nc


def _prep_inputs(inputs):
    import ml_dtypes
    F16 = np.float16
    pair = np.asarray(inputs["pair"], dtype=np.float32)
    bias = np.asarray(inputs["bias"], dtype=np.float32)
    mask = np.asarray(inputs["mask"])
    assert bool(mask.all()), "kernel specialized for all-ones mask"
    lnpw = np.asarray(inputs["ln_pair_w"], np.float32)
    lnpb = np.asarray(inputs["ln_pair_b"], np.float32)
    lnbw = np.asarray(inputs["ln_bias_w"], np.float32)
    lnbb = np.asarray(inputs["ln_bias_b"], np.float32)
    assert np.abs(lnpb).max() == 0.0 and np.abs(lnbb).max() == 0.0, \
        "kernel specialized for zero LN biases"
    Wq = np.asarray(inputs["Wq"], np.float32)
    Wk = np.asarray(inputs["Wk"], np.float32)
    Wv = np.asarray(inputs["Wv"], np.float32)
    Wb = np.asarray(inputs["Wb"], np.float32)
    Wg = np.asarray(inputs["Wg"], np.float32)
    bg = np.asarray(inputs["bg"], np.float32)
    Wo = np.asarray(inputs["Wo"], np.float32)
    bo = np.asarray(inputs["bo"], np.float32)

    pairT = np.ascontiguousarray(pair[0].transpose(1, 0, 2))   # X_raw[n, m, c]
    biasT = np.ascontiguousarray(bias[0].transpose(1, 0, 2))

    # permutation for einsum output partitions: P = d*4 + hq (per half)
    perm = np.empty(HD, np.int64)
    for half in range(2):
        for hq in range(4):
            for d_ in range(DH):
                perm[half * 128 + d_ * 4 + hq] = (4 * half + hq) * DH + d_
    wg_perm = (lnpw[:, None] * Wg)[:, perm]
    ones32 = np.full((D, 32), 1.0 / 128.0, np.float32)
    w_all = np.concatenate([
        (lnpw[:, None] * Wq) / math.sqrt(DH),
        (lnpw[:, None] * Wk) / math.sqrt(L),
        lnpw[:, None] * Wv,
        wg_perm,
        np.tile(lnbw[:, None] * Wb, (1, 4)),
        ones32,
    ], axis=1)
    wo_p = Wo[perm, :]
    wo2 = np.concatenate([wo_p[0:128, :], wo_p[128:256, :]], axis=1)
    bg_perm = bg[perm]

    base = {
        "w_all": np.ascontiguousarray(w_all.astype(F16)),
        "wo2": np.ascontiguousarray(wo2.astype(F16)),
        "bo": bo.reshape(D, 1).copy(),
        "bg2": np.ascontiguousarray(bg_perm.reshape(2, 128).T),
        "ident": np.eye(128, dtype=np.float32).astype(F16),
    }
    in_maps = []
    for c in range(N_CORES):
        m = dict(base)
        # pr[chan, r*2048 + g*256 + i] = X_raw[32c + 4g + r, i, chan]
        pr_rows = pairT[32 * c:32 * c + 32]            # [n_local, i, chan]
        pr_rgi = pr_rows.reshape(8, 4, L, D).transpose(1, 0, 2, 3)  # r,g,i,c
        m["pr"] = np.ascontiguousarray(
            pr_rgi.reshape(NTOK, D).T.astype(F16))
        # pc[chan, g*1024 + r*256 + i] = X_raw[i, 32c + 4g + r, chan]
        pc_cols = pairT[:, 32 * c:32 * c + 32]         # [i, k_local, chan]
        pc_gri = pc_cols.transpose(1, 0, 2).reshape(8, 4, L, D) \
            .transpose(0, 1, 2, 3)                     # (4g+r) major
        # k_local = 4g + r -> index [g, r]: k_local axis is (g*4 + ... wait
        pc_kli = pc_cols.transpose(1, 0, 2)            # [k_local, i, chan]
        pc_gr = pc_kli.reshape(8, 4, L, D)             # [g, r, i, c] k=4g+r
        m["pc"] = np.ascontiguousarray(
            pc_gr.reshape(NTOK, D).T.astype(F16))
        # br[chan, s*256 + j] = biasT[32c + s, j, chan]
        m["br"] = np.ascontiguousarray(
            biasT[32 * c:32 * c + 32].reshape(NTOK, D).T.astype(F16))
        in_maps.append(m)
    return in_maps


def _sharded_fn(nc):
    """Build (once) a cached jitted shard_map callable for the program."""
    if "fn" in _cache:
        return _cache["fn"]
    import jax
    import numpy as _np
    from jax.sharding import Mesh, PartitionSpec
    from jax.experimental.shard_map import shard_map
    from concourse import mybir
    from concourse import bass2jax as b2j

    b2j.install_neuronx_cc_hook()
    pid_name = (nc.partition_id_tensor.name
                if nc.partition_id_tensor is not None else None)
    in_names, out_names, out_shapes, out_dtypes = [], [], [], []
    for alloc in nc.m.functions[0].allocations:
        if not isinstance(alloc, mybir.MemoryLocationSet):
            continue
        name = alloc.memorylocations[0].name
        if alloc.kind == "ExternalInput":
            if name == pid_name:
                continue
            in_names.append(name)
        elif alloc.kind == "ExternalOutput":
            out_names.append(name)
            out_shapes.append(tuple(alloc.tensor_shape))
            out_dtypes.append(mybir.dt.np(alloc.dtype))
    n_params = len(in_names)
    n_outs = len(out_names)
    out_avals = [jax.core.ShapedArray(s, d)
                 for s, d in zip(out_shapes, out_dtypes)]
    all_names = in_names + out_names
    if pid_name is not None:
        all_names = all_names + [pid_name]

    def _body(*args):
        ops = list(args)
        if pid_name is not None:
            ops.append(b2j.partition_id_tensor())
        outs = b2j._bass_exec_p.bind(
            *ops,
            out_avals=tuple(out_avals),
            in_names=tuple(all_names),
            out_names=tuple(out_names),
            lowering_input_output_aliases=(),
            sim_require_finite=True,
            sim_require_nnan=True,
            nc=nc,
        )
        return tuple(outs)

    devices = jax.devices()[:N_CORES]
    mesh = Mesh(_np.asarray(devices), ("core",))
    in_specs = (PartitionSpec("core"),) * (n_params + n_outs)
    out_specs = (PartitionSpec("core"),) * n_outs
    donate = tuple(range(n_params, n_params + n_outs))
    fn = jax.jit(
        shard_map(_body, mesh=mesh, in_specs=in_specs, out_specs=out_specs,
                  check_rep=False),
        donate_argnums=donate, keep_unused=True)
    _cache["fn"] = (fn, in_names, out_names, out_shapes, out_dtypes)
    return _cache["fn"]


def kernel(**inputs):
    nc = _build()
    in_maps = _prep_inputs(inputs)
    fn, in_names, out_names, out_shapes, out_dtypes = _sharded_fn(nc)
    concat_in = [np.concatenate([in_maps[c][n] for c in range(N_CORES)],
                                axis=0)
                 for n in in_names]
    concat_zeros = [np.zeros((N_CORES * s[0], *s[1:]), d)
                    for s, d in zip(out_shapes, out_dtypes)]
    out_arrs = fn(*concat_in, *concat_zeros)
    oc_all = np.asarray(out_arrs[out_names.index("out")]) \
        .reshape(N_CORES, D, NTOK).astype(np.float32)
    out = np.empty((1, L, L, D), dtype=np.float32)
    for c in range(N_CORES):
        # col = g*1024 + kl*256 + i ; k_local = 4g + kl
        oc = oc_all[c].reshape(D, 8, 4, L)       # [D, g, kl, i]
        out[0, 32 * c:32 * c + 32] = \
            oc.transpose(1, 2, 3, 0).reshape(NS, L, D)
    return out


if __name__ == "__main__":
    _build()
    print("build ok")


# revision 38
# speedup vs baseline: 1.2041x; 1.0317x over previous
"""Trainium2 Bass kernel for BiasedAxialAttention (tied row attention), 8-core SPMD.

Math (reference, in the transposed frame X = LN(pairT), pairT[a,b,:] = pair[0,b,a,:]):
    q,k,v = X@Wq,Wk,Wv (scaled);  b = LN(biasT)@Wb
    g = sigmoid(X@Wg + bg)
    logits[i,j,h] = sum_{n,d} q[n,i,h,d] k[n,j,h,d] + b[i,j,h]
    attn = softmax_j(logits);  out[i,k,(h,d)] = sum_j attn[i,j,h] v[k,j,h,d]
    out = (g * out) @ Wo + bo;  final[k,i,:] = out[i,k,:]

Sharding (core c of 8): rows R_c = [32c, 32c+32) of X are both the tied
contraction rows (n) and the core's output columns (k). Partial logits are
AllReduduced in fp16 (the bias b is folded into the partials pre-AR, so no
AllGather is needed).

Host ships activations fp16, channel-major (pre-transposed):
    pr[chan, r*2048 + g*256 + i] = X_raw[32c + 4g + r, i, chan]
    pc[chan, g*1024 + r*256 + i] = X_raw[i, 32c + 4g + r, chan]
    br[chan, s*256 + j]          = biasT_raw[32c + s, j, chan]
LN runs on-device: per-token sums via ones-matmuls (replicated over 32-row
PSUM groups via tile_position), finishing on a [128,64] reshape, then the
(r | -m*r) row is partition-broadcast (GPSIMD) and applied as two
tensor_tensor passes. LN gamma and all static scales are folded into the
weights on the host; LN beta must be zero and the mask all-ones (asserted).
"""
import os
import sys

for _p in ("/opt/trn_rl_repo", "/root/.axon_site/_ro/trn_rl_repo"):
    if os.path.isdir(_p) and _p not in sys.path:
        sys.path.append(_p)

import math
import numpy as np

N_CORES = 8
L = 256
D = 128
H = 8
DH = 32
HD = H * DH          # 256
NS = L // N_CORES    # 32
NTOK = NS * L        # 8192
EPS = 1e-5

_cache = {}


def _build():
    if "nc" in _cache:
        return _cache["nc"]
    from contextlib import ExitStack

    import concourse.bacc as bacc
    import concourse.bass as cbass
    import concourse.tile as tile
    from concourse import mybir

    F32 = mybir.dt.float32
    F16 = mybir.dt.float16
    AF = mybir.ActivationFunctionType
    ALU = mybir.AluOpType

    nc = bacc.Bacc("TRN2", target_bir_lowering=False, debug=False,
                   num_devices=N_CORES)

    ei = dict(kind="ExternalInput")
    pr_d = nc.dram_tensor("pr", [D, NTOK], F16, **ei)
    pc_d = nc.dram_tensor("pc", [D, NTOK], F16, **ei)
    br_d = nc.dram_tensor("br", [D, NTOK], F16, **ei)
    # w_all: [D, 4*HD + 32 + 32]: wq|wk|wv|wg|wb_x4|ones32 (gamma, scales folded)
    WCOL = 4 * HD + 32 + 32
    wall_d = nc.dram_tensor("w_all", [D, WCOL], F16, **ei)
    wo2_d = nc.dram_tensor("wo2", [128, 2 * D], F16, **ei)
    bo_d = nc.dram_tensor("bo", [D, 1], F32, **ei)
    bg_d = nc.dram_tensor("bg2", [128, 2], F32, **ei)
    id_d = nc.dram_tensor("ident", [128, 128], F16, **ei)

    out_d = nc.dram_tensor("out", [D, NTOK], F16, kind="ExternalOutput")

    with tile.TileContext(nc) as tc, ExitStack() as ctx:
        singles = ctx.enter_context(tc.tile_pool(name="singles", bufs=1))
        small = ctx.enter_context(tc.tile_pool(name="small", bufs=1))
        p2 = ctx.enter_context(tc.tile_pool(name="p2", bufs=2))
        big = ctx.enter_context(tc.tile_pool(name="big", bufs=1))
        ps = ctx.enter_context(tc.tile_pool(name="ps", bufs=2, space="PSUM"))
        ps_st = ctx.enter_context(tc.tile_pool(name="ps_st", bufs=1,
                                               space="PSUM"))
        ps_t = ctx.enter_context(tc.tile_pool(name="ps_t", bufs=2,
                                              space="PSUM"))
        dram = ctx.enter_context(tc.tile_pool(name="dram", bufs=1,
                                              space="DRAM"))

        # ---------------- constants ----------------
        w_sb = singles.tile([128, WCOL], F16, tag="w_sb")
        nc.scalar.dma_start(w_sb[:], wall_d.ap())
        wq = w_sb[:, 0:HD]
        wk = w_sb[:, HD:2 * HD]
        wv = w_sb[:, 2 * HD:3 * HD]
        wg = w_sb[:, 3 * HD:4 * HD]
        wb = w_sb[:, 4 * HD:4 * HD + 32]
        ones32 = w_sb[:, 4 * HD + 32:4 * HD + 64]
        wo_sb = singles.tile([128, 2 * D], F16, tag="wo_sb")
        nc.scalar.dma_start(wo_sb[:], wo2_d.ap())
        bo_sb = singles.tile([128, 1], F32, tag="bo")
        nc.scalar.dma_start(bo_sb[:], bo_d.ap())
        bg_sb = singles.tile([128, 2], F32, tag="bg")
        nc.scalar.dma_start(bg_sb[:], bg_d.ap())
        ident = singles.tile([128, 128], F16, tag="ident")
        nc.scalar.dma_start(ident[:], id_d.ap())
        eps_t = singles.tile([128, 1], F32, tag="eps")
        nc.vector.memset(eps_t[:], EPS)

        # ---------------- input loads (4 chunks each) -----------------------
        xt_br = big.tile([128, NTOK], F16, tag="xt_br")
        xt_pr = big.tile([128, NTOK], F16, tag="xt_pr")
        xt_pc = big.tile([128, NTOK], F16, tag="xt_pc")
        for t4 in range(4):
            sl = slice(2048 * t4, 2048 * (t4 + 1))
            nc.sync.dma_start(xt_br[:, sl], br_d.ap()[:, sl])
        for t4 in range(4):
            sl = slice(2048 * t4, 2048 * (t4 + 1))
            nc.sync.dma_start(xt_pr[:, sl], pr_d.ap()[:, sl])
        for t4 in range(4):
            sl = slice(2048 * t4, 2048 * (t4 + 1))
            nc.scalar.dma_start(xt_pc[:, sl], pc_d.ap()[:, sl])

        # ---------------- DRAM collective tiles -----------------------------
        ar_in = [dram.tile([4, 2, 128, L], F16, tag=f"ar_in{i}",
                           name=f"ar_in{i}") for i in range(2)]
        ar_out = [dram.tile([4, 2, 128, L], F16, tag=f"ar_out{i}",
                            name=f"ar_out{i}", addr_space="Shared")
                  for i in range(2)]

        # =============== layernorm: stats + normalize ========================
        # stats: Sx(t) and Sq(t) (pre-scaled by 1/128) for all 8192 tokens.
        # ones32-matmul replicates each chunk's sums over a 32-row PSUM group;
        # chunk j -> rows [32*(j//4), +32), cols [512*(j%4), +512).
        def ln_stats_a(xt, nm):
            sq = big.tile([128, NTOK], F16, tag="ppq", name="sq")
            for blk in range(4):
                nc.vector.tensor_tensor(
                    out=sq[:, 2048 * blk:2048 * (blk + 1)],
                    in0=xt[:, 2048 * blk:2048 * (blk + 1)],
                    in1=xt[:, 2048 * blk:2048 * (blk + 1)], op=ALU.mult)
            m128 = small.tile([128, 64], F16, tag=f"m128_{nm}",
                              name=f"m128_{nm}")
            e128 = small.tile([128, 64], F16, tag=f"e128_{nm}",
                              name=f"e128_{nm}")
            for st in range(2):
                mps = ps_st.tile([128, 2048], F32, tag="stat",
                                 name=f"mps{st}")
                srcbuf = xt if st == 0 else sq
                for j in range(16):   # 512-token chunks
                    rg, slot = j // 4, j % 4
                    nc.tensor.matmul(
                        mps[32 * rg:32 * (rg + 1),
                            512 * slot:512 * (slot + 1)], ones32,
                        srcbuf[:, 512 * j:512 * (j + 1)],
                        start=True, stop=True, tile_position=(0, 32 * rg))
                # evac full replicated tile (engine partition access must
                # be contiguous); the DMA then picks rows {0,32,64,96} and
                # reshapes straight to [128, 64] (t = 64p + f), SBUF->SBUF.
                sx4 = small.tile([128, 2048], F16, tag="sx4",
                                 name=f"sx4_{nm}{st}")
                if st == 0:
                    nc.vector.tensor_copy(sx4[:], mps[:])
                else:
                    nc.scalar.activation(out=sx4[:], in_=mps[:], func=AF.Copy)
                st_d = dram.tile([4, 2048], F16, tag=f"st_d_{nm}{st}",
                                 name=f"st_d_{nm}{st}")
                nc.sync.dma_start(
                    st_d[:],
                    sx4[:].rearrange("(a b) f -> a b f", b=32)[:, 0, :])
                dst = m128 if st == 0 else e128
                nc.scalar.dma_start(
                    dst[:], st_d[:].rearrange("a (b f) -> (a b) f", f=64))
            return m128, e128

        def ln_stats_b(me, nm):
            m128, e128 = me
            # finishing: var = E[x^2] - m^2 ; r = 1/sqrt(var+eps); nmr = -m*r
            var = small.tile([128, 64], F16, tag="var")
            nc.vector.scalar_tensor_tensor(
                out=var[:], in0=m128[:], scalar=-1.0, in1=m128[:],
                op0=ALU.mult, op1=ALU.mult)            # -m^2
            nc.vector.tensor_tensor(out=var[:], in0=e128[:], in1=var[:],
                                    op=ALU.add)        # E[x^2]-m^2
            std = small.tile([128, 64], F32, tag="std")
            nc.scalar.activation(out=std[:], in_=var[:], func=AF.Sqrt,
                                 bias=eps_t[:], scale=1.0)
            rec = small.tile([128, 64], F32, tag="rec")
            nc.vector.reciprocal(out=rec[:], in_=std[:])
            r16 = small.tile([128, 64], F16, tag="r16")
            nc.vector.tensor_copy(r16[:], rec[:])
            nmr = small.tile([128, 64], F16, tag="nmr")
            nc.vector.scalar_tensor_tensor(
                out=nmr[:], in0=m128[:], scalar=-1.0, in1=rec[:],
                op0=ALU.mult, op1=ALU.mult)
            # rows to DRAM (broadcast source); t = 64*p + f
            rm_d = dram.tile([2, 128, 64], F16, tag=f"rm_d_{nm}",
                             name=f"rm_d_{nm}")
            nc.sync.dma_start(rm_d[0], r16[:])
            nc.sync.dma_start(rm_d[1], nmr[:])
            return rm_d

        def ln_norm(xt, rm_d):
            # broadcast r / -m*r rows from DRAM (stride-0 partition source)
            # into [128, NTOK] SBUF tiles (chunked, on separate queues so the
            # TT passes start early), applied as two TT passes.
            for st, op in ((0, ALU.mult), (1, ALU.add)):
                for hh in range(2):
                    bc = p2.tile([128, 4096], F16, tag="rmb",
                                 name=f"rmb{st}{hh}")
                    eng = nc.scalar if st == 0 else nc.gpsimd
                    eng.dma_start(
                        bc[:],
                        rm_d[st].rearrange("p f -> (p f)")
                        [4096 * hh:4096 * (hh + 1)].partition_broadcast(128))
                    for b2 in range(2):
                        sl = slice(4096 * hh + 2048 * b2,
                                   4096 * hh + 2048 * (b2 + 1))
                        nc.vector.tensor_tensor(
                            out=xt[:, sl], in0=xt[:, sl],
                            in1=bc[:, 2048 * b2:2048 * (b2 + 1)], op=op)

        me_br = ln_stats_a(xt_br, "br")
        me_pr = ln_stats_a(xt_pr, "pr")
        me_pc = ln_stats_a(xt_pc, "pc")
        rm_br = ln_stats_b(me_br, "br")
        rm_pr = ln_stats_b(me_pr, "pr")
        rm_pc = ln_stats_b(me_pc, "pc")
        ln_norm(xt_br, rm_br)
        ln_norm(xt_pr, rm_pr)
        ln_norm(xt_pc, rm_pc)

        # ====== b projection -> b_d DRAM [4 t4, 8 h, (8 s8, 256 j)] =========
        # b[h, s = 8*t4 + s8, j] lives at b_d[t4, h, (s8, j)]; the pre-AR
        # accumulate DMAs read it DRAM->DRAM with the CCE add.
        b_d = dram.tile([4, 8, 2048], F16, tag="b_d", name="b_d")
        for t4 in range(4):
            pb = ps_st.tile([32, 2048], F32, tag="stat", name="b_ps")
            for j in range(4):
                nc.tensor.matmul(pb[:, 512 * j:512 * (j + 1)], wb,
                                 xt_br[:, 2048 * t4 + 512 * j:
                                       2048 * t4 + 512 * (j + 1)],
                                 start=True, stop=True)
            bh = p2.tile([8, 2048], F16, tag="b_hd", name=f"bh{t4}")
            if t4 % 2 == 0:
                nc.vector.tensor_copy(bh[:], pb[0:8, :])
            else:
                nc.scalar.activation(out=bh[:], in_=pb[0:8, :], func=AF.Copy)
            nc.scalar.dma_start(b_d[t4], bh[:])

        # =============== q/k projection + pack + logits per half =============
        ppq = big.tile([128, NTOK], F16, tag="ppq")
        ppk = big.tile([128, NTOK], F16, tag="ppk")
        pkq_all = big.tile([128, 4, 8, L], F16, tag="xt_br", name="pkq_all")
        pkk_all = big.tile([128, 4, 8, L], F16, tag="pkk_all",
                           name="pkk_all")
        pkq = [pkq_all[:, i] for i in range(4)]
        pkk = [pkk_all[:, i] for i in range(4)]

        def evac(dst, src, k):
            if k % 3 in (0, 1):
                nc.scalar.activation(out=dst, in_=src, func=AF.Copy)
            else:
                nc.vector.tensor_copy(dst, src)

        def proj_half(w_, half, dst):
            for ch in range(16):
                pm = ps.tile([128, 512], F32, tag="mm")
                nc.tensor.matmul(pm[:], w_[:, 128 * half:128 * (half + 1)],
                                 dst[1][:, 512 * ch:512 * (ch + 1)],
                                 start=True, stop=True)
                evac(dst[0][:, 512 * ch:512 * (ch + 1)], pm[:], ch)

        ls_stage = [p2.tile([128, 4, 2, L], F16, tag="lss", name=f"lss{i}")
                    for i in range(2)]

        def qk_half(half):
            proj_half(wq, half, (ppq, xt_pr))
            proj_half(wk, half, (ppk, xt_pr))
            for hq in range(4):
                for r in range(4):
                    nc.sync.dma_start(
                        pkq[hq][32 * r:32 * (r + 1), :, :]
                        .rearrange("d g i -> d (g i)"),
                        ppq[32 * hq:32 * (hq + 1),
                            2048 * r:2048 * (r + 1)])
                    nc.scalar.dma_start(
                        pkk[hq][32 * r:32 * (r + 1), :, :]
                        .rearrange("d g i -> d (g i)"),
                        ppk[32 * hq:32 * (hq + 1),
                            2048 * r:2048 * (r + 1)])
            lss = ls_stage[half]
            for hq in range(4):
                pl = ps.tile([128, 2, L], F32, tag="mm", name="pl")
                for ih in range(2):
                    for g in range(8):
                        nc.tensor.matmul(
                            pl[:, ih, :],
                            pkq[hq][:, g, 128 * ih:128 * (ih + 1)],
                            pkk[hq][:, g, :], start=(g == 0), stop=(g == 7))
                evac(lss[:, hq, :, :].rearrange("p a b -> p (a b)"),
                     pl[:].rearrange("p a b -> p (a b)"), hq)
            return lss

        # fold b into the partial logits pre-AR: core c owns global rows
        # i = 32*(c%4) + s in half ih_c = c//4; DMA-accumulate b_t into
        # ar_in at a core-id-dependent DRAM offset (CCE add on the DMA).
        if not os.environ.get("KNOB_NO_BADD"):
            pid = nc.gpsimd.partition_id()
            ih_reg = pid // 4
            pc_reg = (pid % 4) * 32

        def run_half(half):
            lss = qk_half(half)
            nc.sync.dma_start(
                ar_in[half][:].rearrange("hq ih p j -> p hq ih j"), lss[:])
            if not os.environ.get("KNOB_NO_BADD"):
                for t4 in range(4):
                    nc.gpsimd.dma_start(
                        ar_in[half][:, cbass.ds(ih_reg, 1),
                                    cbass.ds(pc_reg + 8 * t4, 8), :],
                        b_d[t4, 4 * half:4 * (half + 1), :]
                        .rearrange("h (s j) -> h s j", s=8),
                        accum_op=ALU.add)
            if os.environ.get("KNOB_NO_AR"):
                nc.scalar.dma_start(ar_out[half][:], ar_in[half][:])
            else:
                nc.gpsimd.collective_compute(
                    "AllReduce", ALU.add,
                    replica_groups=[list(range(N_CORES))],
                    ins=[ar_in[half].opt()], outs=[ar_out[half].opt()],
                )

        run_half(0)
        run_half(1)

        # =============== v projection -> vkg[g] ==============================
        vkg_all = big.tile([128, 8, 2, 8, 4, DH], F16, tag="vkg_all",
                           name="vkg_all")
        vkg = [vkg_all[:, g] for g in range(8)]
        kv = 0
        for g in range(8):
            for r in range(4):
                for ih in range(2):
                    coff = 2048 * r + 256 * g + 128 * ih
                    pv = ps.tile([128, 256], F32, tag="mm", name="pv")
                    nc.tensor.matmul(pv[:], xt_pr[:, coff:coff + 128], wv,
                                     start=True, stop=True)
                    dst = vkg[g][:, ih, :, r, :]
                    src = pv[:].rearrange("p (h d) -> p h d", h=8)
                    evac(dst, src, kv)
                    kv += 1

        # =============== gate projection (sigmoid) ===========================
        gs = [big.tile([128, NTOK], F16, tag="ppq", name="gs0"),
              big.tile([128, NTOK], F16, tag="ppk", name="gs1")]
        for q in range(2):
            for ch in range(16):
                pg = ps.tile([128, 512], F32, tag="mm", name="pg")
                nc.tensor.matmul(pg[:], wg[:, 128 * q:128 * (q + 1)],
                                 xt_pc[:, 512 * ch:512 * (ch + 1)],
                                 start=True, stop=True)
                nc.scalar.activation(
                    out=gs[q][:, 512 * ch:512 * (ch + 1)], in_=pg[:],
                    func=AF.Sigmoid, bias=bg_sb[:, q:q + 1], scale=1.0)

        # =============== softmax per half -> at_t ============================
        at_t = big.tile([128, 2, 8, 2, 128], F16, tag="xt_br", name="at_t")
        atn = p2.tile([128, 4, 2, L], F16, tag="lss", name="atn")

        def softmax(half):
            lsb = p2.tile([128, 4, 2, L], F16, tag="lsb", bufs=1)
            nc.sync.dma_start(
                lsb[:], ar_out[half][:].rearrange("hq ih p j -> p hq ih j"))
            sm_s = small.tile([128, 8], F32, tag="sm_s")
            nmx = small.tile([128, 8], F32, tag="nmx")
            for hq in range(4):
                for ih in range(2):
                    cc = 2 * hq + ih
                    nc.vector.tensor_reduce(
                        out=nmx[:, cc:cc + 1], in_=lsb[:, hq, ih, :],
                        axis=mybir.AxisListType.X, op=ALU.max, negate=True)
            for hq in range(4):
                for ih in range(2):
                    cc = 2 * hq + ih
                    nc.scalar.activation(
                        out=atn[:, hq, ih, :], in_=lsb[:, hq, ih, :],
                        func=AF.Exp, bias=nmx[:, cc:cc + 1], scale=1.0,
                        accum_out=sm_s[:, cc:cc + 1])
            sm_r = small.tile([128, 8], F32, tag="sm_r")
            nc.vector.reciprocal(out=sm_r[:], in_=sm_s[:])
            for hq in range(4):
                h = 4 * half + hq
                for ih in range(2):
                    nc.vector.tensor_scalar(
                        out=atn[:, hq, ih, :], in0=atn[:, hq, ih, :],
                        scalar1=sm_r[:, 2 * hq + ih:2 * hq + ih + 1],
                        scalar2=None, op0=ALU.mult)
                pt = ps_t.tile([128, 2, 2, 128], F16, tag="pt")
                for ih in range(2):
                    for jh in range(2):
                        nc.tensor.transpose(
                            pt[:, ih, jh, :],
                            atn[:, hq, ih, 128 * jh:128 * (jh + 1)],
                            ident[:])
                evac(at_t[:, :, h, :, :].rearrange("p jh ih i -> p ih jh i"),
                     pt[:], hq)

        softmax(0)

        # ======= einsum phase A (heads 0-3 -> even h' -> okg q=0) ==========
        # runs inside the AR1 shadow; phase B (heads 4-7, q=1) follows
        # softmax(1), then the Wo projection consumes both.
        okg_all = big.tile([128, 8, 2, 4, L], F16, tag="xt_pr",
                           name="okg_all")

        def ein_half(g, hb):
            ein = p2.tile([128, 8, L], F16, tag="ein", name=f"ein{g}_{hb}")
            for hp in (2 * hb, 2 * hb + 1):
                hps = hp if hb == 0 else hp - 2
                po = ps.tile([128, 2, L], F32, tag="mm", name="po")
                for hh in range(2):
                    h = (4 * hb) + 2 * hps + hh if False else 2 * hp + hh
                    for jh in range(2):
                        nc.tensor.matmul(
                            po[:, hh, :],
                            vkg[g][:, jh, h, :, :]
                            .rearrange("p kl d -> p (kl d)"),
                            at_t[:, jh, h, :, :]
                            .rearrange("p ih i -> p (ih i)"),
                            start=(jh == 0), stop=(jh == 1))
                dst = ein[:].rearrange("p (k par) i -> p par k i", par=2)[
                    :, hp // 2, 2 * (hp % 2):2 * (hp % 2) + 2, :]
                evac(dst, po[:], hp + g)
            q = hb
            for kl in range(4):
                eng = (nc.sync, nc.scalar, nc.gpsimd, nc.gpsimd)[kl]
                eng.dma_start(
                    okg_all[:, g, q, kl, :],
                    ein[32 * kl:32 * (kl + 1), :, :]
                    .rearrange("d (hq q) i -> d q hq i", q=2)[:, q, :, :])
            nc.vector.tensor_tensor(
                out=okg_all[:, g, q, :, :].rearrange("p kl i -> p (kl i)"),
                in0=okg_all[:, g, q, :, :].rearrange("p kl i -> p (kl i)"),
                in1=gs[q][:, 1024 * g:1024 * (g + 1)], op=ALU.mult)

        def wo_block(g):
            ot = p2.tile([128, 1024], F16, tag="ot", name=f"ot{g}",
                         bufs=1)
            for cc in range(2):
                pw = ps.tile([128, 512], F32, tag="mm", name="pw")
                nc.tensor.matmul(pw[:], wo_sb[:, 0:D],
                                 okg_all[:, g, 0, 2 * cc:2 * (cc + 1), :]
                                 .rearrange("p kl i -> p (kl i)"),
                                 start=True, stop=False)
                nc.tensor.matmul(pw[:], wo_sb[:, D:2 * D],
                                 okg_all[:, g, 1, 2 * cc:2 * (cc + 1), :]
                                 .rearrange("p kl i -> p (kl i)"),
                                 start=False, stop=True)
                if (g + cc) % 2 == 0:
                    nc.scalar.activation(out=ot[:, 512 * cc:512 * (cc + 1)],
                                         in_=pw[:], func=AF.Identity,
                                         bias=bo_sb[:], scale=1.0)
                else:
                    nc.vector.tensor_scalar(
                        out=ot[:, 512 * cc:512 * (cc + 1)], in0=pw[:],
                        scalar1=bo_sb[:], scalar2=None, op0=ALU.add)
            eng = (nc.sync, nc.scalar, nc.gpsimd)[g % 3]
            eng.dma_start(out_d.ap()[:, 1024 * g:1024 * (g + 1)], ot[:])

        for g in range(8):
            ein_half(g, 0)
        softmax(1)
        for g in range(8):
            ein_half(g, 1)
        for g in range(8):
            wo_block(g)

    nc.compile()
    _cache["nc"] = nc
    return nc


def _prep_inputs(inputs):
    import ml_dtypes
    F16 = np.float16
    pair = np.asarray(inputs["pair"], dtype=np.float32)
    bias = np.asarray(inputs["bias"], dtype=np.float32)
    mask = np.asarray(inputs["mask"])
    assert bool(mask.all()), "kernel specialized for all-ones mask"
    lnpw = np.asarray(inputs["ln_pair_w"], np.float32)
    lnpb = np.asarray(inputs["ln_pair_b"], np.float32)
    lnbw = np.asarray(inputs["ln_bias_w"], np.float32)
    lnbb = np.asarray(inputs["ln_bias_b"], np.float32)
    assert np.abs(lnpb).max() == 0.0 and np.abs(lnbb).max() == 0.0, \
        "kernel specialized for zero LN biases"
    Wq = np.asarray(inputs["Wq"], np.float32)
    Wk = np.asarray(inputs["Wk"], np.float32)
    Wv = np.asarray(inputs["Wv"], np.float32)
    Wb = np.asarray(inputs["Wb"], np.float32)
    Wg = np.asarray(inputs["Wg"], np.float32)
    bg = np.asarray(inputs["bg"], np.float32)
    Wo = np.asarray(inputs["Wo"], np.float32)
    bo = np.asarray(inputs["bo"], np.float32)

    pairT = np.ascontiguousarray(pair[0].transpose(1, 0, 2))   # X_raw[n, m, c]
    biasT = np.ascontiguousarray(bias[0].transpose(1, 0, 2))

    # permutation for einsum output partitions: P = d*4 + hq (per half)
    perm = np.empty(HD, np.int64)
    for half in range(2):
        for hq in range(4):
            for d_ in range(DH):
                perm[half * 128 + d_ * 4 + hq] = (4 * half + hq) * DH + d_
    wg_perm = (lnpw[:, None] * Wg)[:, perm]
    ones32 = np.full((D, 32), 1.0 / 128.0, np.float32)
    w_all = np.concatenate([
        (lnpw[:, None] * Wq) / math.sqrt(DH),
        (lnpw[:, None] * Wk) / math.sqrt(L),
        lnpw[:, None] * Wv,
        wg_perm,
        np.tile(lnbw[:, None] * Wb, (1, 4)),
        ones32,
    ], axis=1)
    wo_p = Wo[perm, :]
    wo2 = np.concatenate([wo_p[0:128, :], wo_p[128:256, :]], axis=1)
    bg_perm = bg[perm]

    base = {
        "w_all": np.ascontiguousarray(w_all.astype(F16)),
        "wo2": np.ascontiguousarray(wo2.astype(F16)),
        "bo": bo.reshape(D, 1).copy(),
        "bg2": np.ascontiguousarray(bg_perm.reshape(2, 128).T),
        "ident": np.eye(128, dtype=np.float32).astype(F16),
    }
    in_maps = []
    for c in range(N_CORES):
        m = dict(base)
        # pr[chan, r*2048 + g*256 + i] = X_raw[32c + 4g + r, i, chan]
        pr_rows = pairT[32 * c:32 * c + 32]            # [n_local, i, chan]
        pr_rgi = pr_rows.reshape(8, 4, L, D).transpose(1, 0, 2, 3)  # r,g,i,c
        m["pr"] = np.ascontiguousarray(
            pr_rgi.reshape(NTOK, D).T.astype(F16))
        # pc[chan, g*1024 + r*256 + i] = X_raw[i, 32c + 4g + r, chan]
        pc_cols = pairT[:, 32 * c:32 * c + 32]         # [i, k_local, chan]
        pc_gri = pc_cols.transpose(1, 0, 2).reshape(8, 4, L, D) \
            .transpose(0, 1, 2, 3)                     # (4g+r) major
        # k_local = 4g + r -> index [g, r]: k_local axis is (g*4 + ... wait
        pc_kli = pc_cols.transpose(1, 0, 2)            # [k_local, i, chan]
        pc_gr = pc_kli.reshape(8, 4, L, D)             # [g, r, i, c] k=4g+r
        m["pc"] = np.ascontiguousarray(
            pc_gr.reshape(NTOK, D).T.astype(F16))
        # br[chan, s*256 + j] = biasT[32c + s, j, chan]
        m["br"] = np.ascontiguousarray(
            biasT[32 * c:32 * c + 32].reshape(NTOK, D).T.astype(F16))
        in_maps.append(m)
    return in_maps


def _sharded_fn(nc):
    """Build (once) a cached jitted shard_map callable for the program."""
    if "fn" in _cache:
        return _cache["fn"]
    import jax
    import numpy as _np
    from jax.sharding import Mesh, PartitionSpec
    from jax.experimental.shard_map import shard_map
    from concourse import mybir
    from concourse import bass2jax as b2j

    b2j.install_neuronx_cc_hook()
    pid_name = (nc.partition_id_tensor.name
                if nc.partition_id_tensor is not None else None)
    in_names, out_names, out_shapes, out_dtypes = [], [], [], []
    for alloc in nc.m.functions[0].allocations:
        if not isinstance(alloc, mybir.MemoryLocationSet):
            continue
        name = alloc.memorylocations[0].name
        if alloc.kind == "ExternalInput":
            if name == pid_name:
                continue
            in_names.append(name)
        elif alloc.kind == "ExternalOutput":
            out_names.append(name)
            out_shapes.append(tuple(alloc.tensor_shape))
            out_dtypes.append(mybir.dt.np(alloc.dtype))
    n_params = len(in_names)
    n_outs = len(out_names)
    out_avals = [jax.core.ShapedArray(s, d)
                 for s, d in zip(out_shapes, out_dtypes)]
    all_names = in_names + out_names
    if pid_name is not None:
        all_names = all_names + [pid_name]

    def _body(*args):
        ops = list(args)
        if pid_name is not None:
            ops.append(b2j.partition_id_tensor())
        outs = b2j._bass_exec_p.bind(
            *ops,
            out_avals=tuple(out_avals),
            in_names=tuple(all_names),
            out_names=tuple(out_names),
            lowering_input_output_aliases=(),
            sim_require_finite=True,
            sim_require_nnan=True,
            nc=nc,
        )
        return tuple(outs)

    devices = jax.devices()[:N_CORES]
    mesh = Mesh(_np.asarray(devices), ("core",))
    in_specs = (PartitionSpec("core"),) * (n_params + n_outs)
    out_specs = (PartitionSpec("core"),) * n_outs
    donate = tuple(range(n_params, n_params + n_outs))
    fn = jax.jit(
        shard_map(_body, mesh=mesh, in_specs=in_specs, out_specs=out_specs,
                  check_rep=False),
        donate_argnums=donate, keep_unused=True)
    _cache["fn"] = (fn, in_names, out_names, out_shapes, out_dtypes)
    return _cache["fn"]


def kernel(**inputs):
    nc = _build()
    in_maps = _prep_inputs(inputs)
    fn, in_names, out_names, out_shapes, out_dtypes = _sharded_fn(nc)
    concat_in = [np.concatenate([in_maps[c][n] for c in range(N_CORES)],
                                axis=0)
                 for n in in_names]
    concat_zeros = [np.zeros((N_CORES * s[0], *s[1:]), d)
                    for s, d in zip(out_shapes, out_dtypes)]
    out_arrs = fn(*concat_in, *concat_zeros)
    oc_all = np.asarray(out_arrs[out_names.index("out")]) \
        .reshape(N_CORES, D, NTOK).astype(np.float32)
    out = np.empty((1, L, L, D), dtype=np.float32)
    for c in range(N_CORES):
        # col = g*1024 + kl*256 + i ; k_local = 4g + kl
        oc = oc_all[c].reshape(D, 8, 4, L)       # [D, g, kl, i]
        out[0, 32 * c:32 * c + 32] = \
            oc.transpose(1, 2, 3, 0).reshape(NS, L, D)
    return out


if __name__ == "__main__":
    _build()
    print("build ok")


# revision 39
# speedup vs baseline: 1.2207x; 1.0138x over previous
"""Trainium2 Bass kernel for BiasedAxialAttention (tied row attention), 8-core SPMD.

Math (reference, in the transposed frame X = LN(pairT), pairT[a,b,:] = pair[0,b,a,:]):
    q,k,v = X@Wq,Wk,Wv (scaled);  b = LN(biasT)@Wb
    g = sigmoid(X@Wg + bg)
    logits[i,j,h] = sum_{n,d} q[n,i,h,d] k[n,j,h,d] + b[i,j,h]
    attn = softmax_j(logits);  out[i,k,(h,d)] = sum_j attn[i,j,h] v[k,j,h,d]
    out = (g * out) @ Wo + bo;  final[k,i,:] = out[i,k,:]

Sharding (core c of 8): rows R_c = [32c, 32c+32) of X are both the tied
contraction rows (n) and the core's output columns (k). Partial logits are
AllReduduced in fp16 (the bias b is folded into the partials pre-AR, so no
AllGather is needed).

Host ships activations fp16, channel-major (pre-transposed):
    pr[chan, r*2048 + g*256 + i] = X_raw[32c + 4g + r, i, chan]
    pc[chan, g*1024 + r*256 + i] = X_raw[i, 32c + 4g + r, chan]
    br[chan, s*256 + j]          = biasT_raw[32c + s, j, chan]
LN runs on-device: per-token sums via ones-matmuls (replicated over 32-row
PSUM groups via tile_position), finishing on a [128,64] reshape, then the
(r | -m*r) row is partition-broadcast (GPSIMD) and applied as two
tensor_tensor passes. LN gamma and all static scales are folded into the
weights on the host; LN beta must be zero and the mask all-ones (asserted).
"""
import os
import sys

for _p in ("/opt/trn_rl_repo", "/root/.axon_site/_ro/trn_rl_repo"):
    if os.path.isdir(_p) and _p not in sys.path:
        sys.path.append(_p)

import math
import numpy as np

N_CORES = 8
L = 256
D = 128
H = 8
DH = 32
HD = H * DH          # 256
NS = L // N_CORES    # 32
NTOK = NS * L        # 8192
EPS = 1e-5

_cache = {}


def _build():
    if "nc" in _cache:
        return _cache["nc"]
    from contextlib import ExitStack

    import concourse.bacc as bacc
    import concourse.bass as cbass
    import concourse.tile as tile
    from concourse import mybir

    F32 = mybir.dt.float32
    F16 = mybir.dt.float16
    AF = mybir.ActivationFunctionType
    ALU = mybir.AluOpType

    nc = bacc.Bacc("TRN2", target_bir_lowering=False, debug=False,
                   num_devices=N_CORES)

    ei = dict(kind="ExternalInput")
    pr_d = nc.dram_tensor("pr", [D, NTOK], F16, **ei)
    pc_d = nc.dram_tensor("pc", [D, NTOK], F16, **ei)
    br_d = nc.dram_tensor("br", [D, NTOK], F16, **ei)
    # w_all: [D, 4*HD + 32 + 32]: wq|wk|wv|wg|wb_x4|ones32 (gamma, scales folded)
    WCOL = 4 * HD + 32 + 32
    wall_d = nc.dram_tensor("w_all", [D, WCOL], F16, **ei)
    wo2_d = nc.dram_tensor("wo2", [128, 2 * D], F16, **ei)
    bo_d = nc.dram_tensor("bo", [D, 1], F32, **ei)
    bg_d = nc.dram_tensor("bg2", [128, 2], F32, **ei)
    id_d = nc.dram_tensor("ident", [128, 128], F16, **ei)

    out_d = nc.dram_tensor("out", [D, NTOK], F16, kind="ExternalOutput")

    with tile.TileContext(nc) as tc, ExitStack() as ctx:
        singles = ctx.enter_context(tc.tile_pool(name="singles", bufs=1))
        small = ctx.enter_context(tc.tile_pool(name="small", bufs=1))
        p2 = ctx.enter_context(tc.tile_pool(name="p2", bufs=2))
        big = ctx.enter_context(tc.tile_pool(name="big", bufs=1))
        ps = ctx.enter_context(tc.tile_pool(name="ps", bufs=2, space="PSUM"))
        ps_st = ctx.enter_context(tc.tile_pool(name="ps_st", bufs=1,
                                               space="PSUM"))
        ps_t = ctx.enter_context(tc.tile_pool(name="ps_t", bufs=2,
                                              space="PSUM"))
        dram = ctx.enter_context(tc.tile_pool(name="dram", bufs=1,
                                              space="DRAM"))

        # ---------------- constants ----------------
        w_sb = singles.tile([128, WCOL], F16, tag="w_sb")
        nc.scalar.dma_start(w_sb[:], wall_d.ap())
        wq = w_sb[:, 0:HD]
        wk = w_sb[:, HD:2 * HD]
        wv = w_sb[:, 2 * HD:3 * HD]
        wg = w_sb[:, 3 * HD:4 * HD]
        wb = w_sb[:, 4 * HD:4 * HD + 32]
        ones32 = w_sb[:, 4 * HD + 32:4 * HD + 64]
        wo_sb = singles.tile([128, 2 * D], F16, tag="wo_sb")
        nc.scalar.dma_start(wo_sb[:], wo2_d.ap())
        bo_sb = singles.tile([128, 1], F32, tag="bo")
        nc.scalar.dma_start(bo_sb[:], bo_d.ap())
        bg_sb = singles.tile([128, 2], F32, tag="bg")
        nc.scalar.dma_start(bg_sb[:], bg_d.ap())
        ident = singles.tile([128, 128], F16, tag="ident")
        nc.scalar.dma_start(ident[:], id_d.ap())
        eps_t = singles.tile([128, 1], F32, tag="eps")
        nc.vector.memset(eps_t[:], EPS)

        # ---------------- input loads (4 chunks each) -----------------------
        xt_br = big.tile([128, NTOK], F16, tag="xt_br")
        xt_pr = big.tile([128, NTOK], F16, tag="xt_pr")
        xt_pc = big.tile([128, NTOK], F16, tag="xt_pc")
        for t4 in range(4):
            sl = slice(2048 * t4, 2048 * (t4 + 1))
            nc.sync.dma_start(xt_br[:, sl], br_d.ap()[:, sl])
        for t4 in range(4):
            sl = slice(2048 * t4, 2048 * (t4 + 1))
            nc.scalar.dma_start(xt_pr[:, sl], pr_d.ap()[:, sl])
        for t4 in range(4):
            sl = slice(2048 * t4, 2048 * (t4 + 1))
            nc.gpsimd.dma_start(xt_pc[:, sl], pc_d.ap()[:, sl])

        # ---------------- DRAM collective tiles -----------------------------
        ar_in = [dram.tile([4, 2, 128, L], F16, tag=f"ar_in{i}",
                           name=f"ar_in{i}") for i in range(2)]
        ar_out = [dram.tile([4, 2, 128, L], F16, tag=f"ar_out{i}",
                            name=f"ar_out{i}", addr_space="Shared")
                  for i in range(2)]

        # =============== layernorm: stats + normalize ========================
        # stats: Sx(t) and Sq(t) (pre-scaled by 1/128) for all 8192 tokens.
        # ones32-matmul replicates each chunk's sums over a 32-row PSUM group;
        # chunk j -> rows [32*(j//4), +32), cols [512*(j%4), +512).
        def ln_stats_a(xt, nm):
            sq = big.tile([128, NTOK], F16, tag="ppq", name="sq")
            for blk in range(4):
                nc.vector.tensor_tensor(
                    out=sq[:, 2048 * blk:2048 * (blk + 1)],
                    in0=xt[:, 2048 * blk:2048 * (blk + 1)],
                    in1=xt[:, 2048 * blk:2048 * (blk + 1)], op=ALU.mult)
            m128 = small.tile([128, 64], F16, tag=f"m128_{nm}",
                              name=f"m128_{nm}")
            e128 = small.tile([128, 64], F16, tag=f"e128_{nm}",
                              name=f"e128_{nm}")
            for st in range(2):
                mps = ps_st.tile([128, 2048], F32, tag="stat",
                                 name=f"mps{st}")
                srcbuf = xt if st == 0 else sq
                for j in range(16):   # 512-token chunks
                    rg, slot = j // 4, j % 4
                    nc.tensor.matmul(
                        mps[32 * rg:32 * (rg + 1),
                            512 * slot:512 * (slot + 1)], ones32,
                        srcbuf[:, 512 * j:512 * (j + 1)],
                        start=True, stop=True, tile_position=(0, 32 * rg))
                # evac full replicated tile (engine partition access must
                # be contiguous); the DMA then picks rows {0,32,64,96} and
                # reshapes straight to [128, 64] (t = 64p + f), SBUF->SBUF.
                sx4 = small.tile([128, 2048], F16, tag="sx4",
                                 name=f"sx4_{nm}{st}")
                if st == 0:
                    nc.vector.tensor_copy(sx4[:], mps[:])
                else:
                    nc.scalar.activation(out=sx4[:], in_=mps[:], func=AF.Copy)
                st_d = dram.tile([4, 2048], F16, tag=f"st_d_{nm}{st}",
                                 name=f"st_d_{nm}{st}")
                nc.sync.dma_start(
                    st_d[:],
                    sx4[:].rearrange("(a b) f -> a b f", b=32)[:, 0, :])
                dst = m128 if st == 0 else e128
                nc.scalar.dma_start(
                    dst[:], st_d[:].rearrange("a (b f) -> (a b) f", f=64))
            return m128, e128

        def ln_stats_b(me, nm):
            m128, e128 = me
            # finishing: var = E[x^2] - m^2 ; r = 1/sqrt(var+eps); nmr = -m*r
            var = small.tile([128, 64], F16, tag="var")
            nc.vector.scalar_tensor_tensor(
                out=var[:], in0=m128[:], scalar=-1.0, in1=m128[:],
                op0=ALU.mult, op1=ALU.mult)            # -m^2
            nc.vector.tensor_tensor(out=var[:], in0=e128[:], in1=var[:],
                                    op=ALU.add)        # E[x^2]-m^2
            std = small.tile([128, 64], F32, tag="std")
            nc.scalar.activation(out=std[:], in_=var[:], func=AF.Sqrt,
                                 bias=eps_t[:], scale=1.0)
            rec = small.tile([128, 64], F32, tag="rec")
            nc.vector.reciprocal(out=rec[:], in_=std[:])
            r16 = small.tile([128, 64], F16, tag="r16")
            nc.vector.tensor_copy(r16[:], rec[:])
            nmr = small.tile([128, 64], F16, tag="nmr")
            nc.vector.scalar_tensor_tensor(
                out=nmr[:], in0=m128[:], scalar=-1.0, in1=rec[:],
                op0=ALU.mult, op1=ALU.mult)
            # rows to DRAM (broadcast source); t = 64*p + f
            rm_d = dram.tile([2, 128, 64], F16, tag=f"rm_d_{nm}",
                             name=f"rm_d_{nm}")
            nc.sync.dma_start(rm_d[0], r16[:])
            nc.sync.dma_start(rm_d[1], nmr[:])
            return rm_d

        def ln_norm(xt, rm_d):
            # broadcast r / -m*r rows from DRAM (stride-0 partition source)
            # into [128, NTOK] SBUF tiles (chunked, on separate queues so the
            # TT passes start early), applied as two TT passes.
            for st, op in ((0, ALU.mult), (1, ALU.add)):
                for hh in range(2):
                    bc = p2.tile([128, 4096], F16, tag="rmb",
                                 name=f"rmb{st}{hh}")
                    eng = nc.scalar if st == 0 else nc.gpsimd
                    eng.dma_start(
                        bc[:],
                        rm_d[st].rearrange("p f -> (p f)")
                        [4096 * hh:4096 * (hh + 1)].partition_broadcast(128))
                    for b2 in range(2):
                        sl = slice(4096 * hh + 2048 * b2,
                                   4096 * hh + 2048 * (b2 + 1))
                        nc.vector.tensor_tensor(
                            out=xt[:, sl], in0=xt[:, sl],
                            in1=bc[:, 2048 * b2:2048 * (b2 + 1)], op=op)

        me_br = ln_stats_a(xt_br, "br")
        me_pr = ln_stats_a(xt_pr, "pr")
        me_pc = ln_stats_a(xt_pc, "pc")
        rm_br = ln_stats_b(me_br, "br")
        rm_pr = ln_stats_b(me_pr, "pr")
        rm_pc = ln_stats_b(me_pc, "pc")
        ln_norm(xt_br, rm_br)
        ln_norm(xt_pr, rm_pr)
        ln_norm(xt_pc, rm_pc)

        # ====== b projection -> b_d DRAM [4 t4, 8 h, (8 s8, 256 j)] =========
        # b[h, s = 8*t4 + s8, j] lives at b_d[t4, h, (s8, j)]; the pre-AR
        # accumulate DMAs read it DRAM->DRAM with the CCE add.
        b_d = dram.tile([4, 8, 2048], F16, tag="b_d", name="b_d")
        for t4 in range(4):
            pb = ps_st.tile([32, 2048], F32, tag="stat", name="b_ps")
            for j in range(4):
                nc.tensor.matmul(pb[:, 512 * j:512 * (j + 1)], wb,
                                 xt_br[:, 2048 * t4 + 512 * j:
                                       2048 * t4 + 512 * (j + 1)],
                                 start=True, stop=True)
            bh = p2.tile([8, 2048], F16, tag="b_hd", name=f"bh{t4}")
            if t4 % 2 == 0:
                nc.vector.tensor_copy(bh[:], pb[0:8, :])
            else:
                nc.scalar.activation(out=bh[:], in_=pb[0:8, :], func=AF.Copy)
            nc.scalar.dma_start(b_d[t4], bh[:])

        # =============== q/k projection + pack + logits per half =============
        ppq = big.tile([128, NTOK], F16, tag="ppq")
        ppk = big.tile([128, NTOK], F16, tag="ppk")
        pkq_all = big.tile([128, 4, 8, L], F16, tag="xt_br", name="pkq_all")
        pkk_all = big.tile([128, 4, 8, L], F16, tag="pkk_all",
                           name="pkk_all")
        pkq = [pkq_all[:, i] for i in range(4)]
        pkk = [pkk_all[:, i] for i in range(4)]

        def evac(dst, src, k):
            if k % 3 in (0, 1):
                nc.scalar.activation(out=dst, in_=src, func=AF.Copy)
            else:
                nc.vector.tensor_copy(dst, src)

        def proj_half(w_, half, dst):
            for ch in range(16):
                pm = ps.tile([128, 512], F32, tag="mm")
                nc.tensor.matmul(pm[:], w_[:, 128 * half:128 * (half + 1)],
                                 dst[1][:, 512 * ch:512 * (ch + 1)],
                                 start=True, stop=True)
                evac(dst[0][:, 512 * ch:512 * (ch + 1)], pm[:], ch)

        ls_stage = [p2.tile([128, 4, 2, L], F16, tag="lss", name=f"lss{i}")
                    for i in range(2)]

        def qk_half(half):
            proj_half(wq, half, (ppq, xt_pr))
            proj_half(wk, half, (ppk, xt_pr))
            for hq in range(4):
                for r in range(4):
                    nc.sync.dma_start(
                        pkq[hq][32 * r:32 * (r + 1), :, :]
                        .rearrange("d g i -> d (g i)"),
                        ppq[32 * hq:32 * (hq + 1),
                            2048 * r:2048 * (r + 1)])
                    nc.scalar.dma_start(
                        pkk[hq][32 * r:32 * (r + 1), :, :]
                        .rearrange("d g i -> d (g i)"),
                        ppk[32 * hq:32 * (hq + 1),
                            2048 * r:2048 * (r + 1)])
            lss = ls_stage[half]
            for hq in range(4):
                pl = ps.tile([128, 2, L], F32, tag="mm", name="pl")
                for ih in range(2):
                    for g in range(8):
                        nc.tensor.matmul(
                            pl[:, ih, :],
                            pkq[hq][:, g, 128 * ih:128 * (ih + 1)],
                            pkk[hq][:, g, :], start=(g == 0), stop=(g == 7))
                evac(lss[:, hq, :, :].rearrange("p a b -> p (a b)"),
                     pl[:].rearrange("p a b -> p (a b)"), hq)
            return lss

        # fold b into the partial logits pre-AR: core c owns global rows
        # i = 32*(c%4) + s in half ih_c = c//4; DMA-accumulate b_t into
        # ar_in at a core-id-dependent DRAM offset (CCE add on the DMA).
        pid = nc.gpsimd.partition_id()
        ih_reg = pid // 4
        pc_reg = (pid % 4) * 32

        def run_half(half):
            lss = qk_half(half)
            nc.sync.dma_start(
                ar_in[half][:].rearrange("hq ih p j -> p hq ih j"), lss[:])
            for t4 in range(4):
                nc.gpsimd.dma_start(
                    ar_in[half][:, cbass.ds(ih_reg, 1),
                                cbass.ds(pc_reg + 8 * t4, 8), :],
                    b_d[t4, 4 * half:4 * (half + 1), :]
                    .rearrange("h (s j) -> h s j", s=8),
                    accum_op=ALU.add)
            nc.gpsimd.collective_compute(
                "AllReduce", ALU.add,
                replica_groups=[list(range(N_CORES))],
                ins=[ar_in[half].opt()], outs=[ar_out[half].opt()],
            )

        run_half(0)
        run_half(1)

        # =============== v projection -> vkg[g] ==============================
        vkg_all = big.tile([128, 8, 2, 8, 4, DH], F16, tag="vkg_all",
                           name="vkg_all")
        vkg = [vkg_all[:, g] for g in range(8)]
        kv = 0
        for g in range(8):
            for r in range(4):
                for ih in range(2):
                    coff = 2048 * r + 256 * g + 128 * ih
                    pv = ps.tile([128, 256], F32, tag="mm", name="pv")
                    nc.tensor.matmul(pv[:], xt_pr[:, coff:coff + 128], wv,
                                     start=True, stop=True)
                    dst = vkg[g][:, ih, :, r, :]
                    src = pv[:].rearrange("p (h d) -> p h d", h=8)
                    evac(dst, src, kv)
                    kv += 1

        # =============== gate projection (sigmoid) ===========================
        gs = [big.tile([128, NTOK], F16, tag="ppq", name="gs0"),
              big.tile([128, NTOK], F16, tag="ppk", name="gs1")]
        for q in range(2):
            for ch in range(16):
                pg = ps.tile([128, 512], F32, tag="mm", name="pg")
                nc.tensor.matmul(pg[:], wg[:, 128 * q:128 * (q + 1)],
                                 xt_pc[:, 512 * ch:512 * (ch + 1)],
                                 start=True, stop=True)
                nc.scalar.activation(
                    out=gs[q][:, 512 * ch:512 * (ch + 1)], in_=pg[:],
                    func=AF.Sigmoid, bias=bg_sb[:, q:q + 1], scale=1.0)

        # =============== softmax per half -> at_t ============================
        at_t = big.tile([128, 2, 8, 2, 128], F16, tag="xt_br", name="at_t")
        atn = p2.tile([128, 4, 2, L], F16, tag="lss", name="atn")

        def softmax(half):
            lsb = p2.tile([128, 4, 2, L], F16, tag="lsb", bufs=1)
            nc.sync.dma_start(
                lsb[:], ar_out[half][:].rearrange("hq ih p j -> p hq ih j"))
            sm_s = small.tile([128, 8], F32, tag="sm_s")
            nmx = small.tile([128, 8], F32, tag="nmx")
            for hq in range(4):
                for ih in range(2):
                    cc = 2 * hq + ih
                    nc.vector.tensor_reduce(
                        out=nmx[:, cc:cc + 1], in_=lsb[:, hq, ih, :],
                        axis=mybir.AxisListType.X, op=ALU.max, negate=True)
            for hq in range(4):
                for ih in range(2):
                    cc = 2 * hq + ih
                    nc.scalar.activation(
                        out=atn[:, hq, ih, :], in_=lsb[:, hq, ih, :],
                        func=AF.Exp, bias=nmx[:, cc:cc + 1], scale=1.0,
                        accum_out=sm_s[:, cc:cc + 1])
            sm_r = small.tile([128, 8], F32, tag="sm_r")
            nc.vector.reciprocal(out=sm_r[:], in_=sm_s[:])
            for hq in range(4):
                h = 4 * half + hq
                for ih in range(2):
                    nc.vector.tensor_scalar(
                        out=atn[:, hq, ih, :], in0=atn[:, hq, ih, :],
                        scalar1=sm_r[:, 2 * hq + ih:2 * hq + ih + 1],
                        scalar2=None, op0=ALU.mult)
                pt = ps_t.tile([128, 2, 2, 128], F16, tag="pt")
                for ih in range(2):
                    for jh in range(2):
                        nc.tensor.transpose(
                            pt[:, ih, jh, :],
                            atn[:, hq, ih, 128 * jh:128 * (jh + 1)],
                            ident[:])
                evac(at_t[:, :, h, :, :].rearrange("p jh ih i -> p ih jh i"),
                     pt[:], hq)

        softmax(0)

        # ======= einsum phase A (heads 0-3 -> even h' -> okg q=0) ==========
        # runs inside the AR1 shadow; phase B (heads 4-7, q=1) follows
        # softmax(1), then the Wo projection consumes both.
        okg_all = big.tile([128, 8, 2, 4, L], F16, tag="xt_pr",
                           name="okg_all")

        def ein_half(g, hb):
            ein = p2.tile([128, 8, L], F16, tag="ein", name=f"ein{g}_{hb}")
            for hp in (2 * hb, 2 * hb + 1):
                hps = hp if hb == 0 else hp - 2
                po = ps.tile([128, 2, L], F32, tag="mm", name="po")
                for hh in range(2):
                    h = (4 * hb) + 2 * hps + hh if False else 2 * hp + hh
                    for jh in range(2):
                        nc.tensor.matmul(
                            po[:, hh, :],
                            vkg[g][:, jh, h, :, :]
                            .rearrange("p kl d -> p (kl d)"),
                            at_t[:, jh, h, :, :]
                            .rearrange("p ih i -> p (ih i)"),
                            start=(jh == 0), stop=(jh == 1))
                dst = ein[:].rearrange("p (k par) i -> p par k i", par=2)[
                    :, hp // 2, 2 * (hp % 2):2 * (hp % 2) + 2, :]
                evac(dst, po[:], hp + g)
            q = hb
            for kl in range(4):
                eng = (nc.sync, nc.scalar, nc.gpsimd, nc.gpsimd)[kl]
                eng.dma_start(
                    okg_all[:, g, q, kl, :],
                    ein[32 * kl:32 * (kl + 1), :, :]
                    .rearrange("d (hq q) i -> d q hq i", q=2)[:, q, :, :])
            nc.vector.tensor_tensor(
                out=okg_all[:, g, q, :, :].rearrange("p kl i -> p (kl i)"),
                in0=okg_all[:, g, q, :, :].rearrange("p kl i -> p (kl i)"),
                in1=gs[q][:, 1024 * g:1024 * (g + 1)], op=ALU.mult)

        def wo_block(g):
            ot = p2.tile([128, 1024], F16, tag="ot", name=f"ot{g}",
                         bufs=1)
            for cc in range(2):
                pw = ps.tile([128, 512], F32, tag="mm", name="pw")
                nc.tensor.matmul(pw[:], wo_sb[:, 0:D],
                                 okg_all[:, g, 0, 2 * cc:2 * (cc + 1), :]
                                 .rearrange("p kl i -> p (kl i)"),
                                 start=True, stop=False)
                nc.tensor.matmul(pw[:], wo_sb[:, D:2 * D],
                                 okg_all[:, g, 1, 2 * cc:2 * (cc + 1), :]
                                 .rearrange("p kl i -> p (kl i)"),
                                 start=False, stop=True)
                if (g + cc) % 2 == 0:
                    nc.scalar.activation(out=ot[:, 512 * cc:512 * (cc + 1)],
                                         in_=pw[:], func=AF.Identity,
                                         bias=bo_sb[:], scale=1.0)
                else:
                    nc.vector.tensor_scalar(
                        out=ot[:, 512 * cc:512 * (cc + 1)], in0=pw[:],
                        scalar1=bo_sb[:], scalar2=None, op0=ALU.add)
            eng = (nc.sync, nc.scalar, nc.gpsimd)[g % 3]
            eng.dma_start(out_d.ap()[:, 1024 * g:1024 * (g + 1)], ot[:])

        for g in range(8):
            ein_half(g, 0)
        softmax(1)
        for g in range(8):
            ein_half(g, 1)
        for g in range(8):
            wo_block(g)

    nc.compile()
    _cache["nc"] = nc
    return nc


def _prep_inputs(inputs):
    import ml_dtypes
    F16 = np.float16
    pair = np.asarray(inputs["pair"], dtype=np.float32)
    bias = np.asarray(inputs["bias"], dtype=np.float32)
    mask = np.asarray(inputs["mask"])
    assert bool(mask.all()), "kernel specialized for all-ones mask"
    lnpw = np.asarray(inputs["ln_pair_w"], np.float32)
    lnpb = np.asarray(inputs["ln_pair_b"], np.float32)
    lnbw = np.asarray(inputs["ln_bias_w"], np.float32)
    lnbb = np.asarray(inputs["ln_bias_b"], np.float32)
    assert np.abs(lnpb).max() == 0.0 and np.abs(lnbb).max() == 0.0, \
        "kernel specialized for zero LN biases"
    Wq = np.asarray(inputs["Wq"], np.float32)
    Wk = np.asarray(inputs["Wk"], np.float32)
    Wv = np.asarray(inputs["Wv"], np.float32)
    Wb = np.asarray(inputs["Wb"], np.float32)
    Wg = np.asarray(inputs["Wg"], np.float32)
    bg = np.asarray(inputs["bg"], np.float32)
    Wo = np.asarray(inputs["Wo"], np.float32)
    bo = np.asarray(inputs["bo"], np.float32)

    pairT = np.ascontiguousarray(pair[0].transpose(1, 0, 2))   # X_raw[n, m, c]
    biasT = np.ascontiguousarray(bias[0].transpose(1, 0, 2))

    # permutation for einsum output partitions: P = d*4 + hq (per half)
    perm = np.empty(HD, np.int64)
    for half in range(2):
        for hq in range(4):
            for d_ in range(DH):
                perm[half * 128 + d_ * 4 + hq] = (4 * half + hq) * DH + d_
    wg_perm = (lnpw[:, None] * Wg)[:, perm]
    ones32 = np.full((D, 32), 1.0 / 128.0, np.float32)
    w_all = np.concatenate([
        (lnpw[:, None] * Wq) / math.sqrt(DH),
        (lnpw[:, None] * Wk) / math.sqrt(L),
        lnpw[:, None] * Wv,
        wg_perm,
        np.tile(lnbw[:, None] * Wb, (1, 4)),
        ones32,
    ], axis=1)
    wo_p = Wo[perm, :]
    wo2 = np.concatenate([wo_p[0:128, :], wo_p[128:256, :]], axis=1)
    bg_perm = bg[perm]

    base = {
        "w_all": np.ascontiguousarray(w_all.astype(F16)),
        "wo2": np.ascontiguousarray(wo2.astype(F16)),
        "bo": bo.reshape(D, 1).copy(),
        "bg2": np.ascontiguousarray(bg_perm.reshape(2, 128).T),
        "ident": np.eye(128, dtype=np.float32).astype(F16),
    }
    in_maps = []
    for c in range(N_CORES):
        m = dict(base)
        # pr[chan, r*2048 + g*256 + i] = X_raw[32c + 4g + r, i, chan]
        pr_rows = pairT[32 * c:32 * c + 32]            # [n_local, i, chan]
        pr_rgi = pr_rows.reshape(8, 4, L, D).transpose(1, 0, 2, 3)  # r,g,i,c
        m["pr"] = np.ascontiguousarray(
            pr_rgi.reshape(NTOK, D).T.astype(F16))
        # pc[chan, g*1024 + r*256 + i] = X_raw[i, 32c + 4g + r, chan]
        pc_cols = pairT[:, 32 * c:32 * c + 32]         # [i, k_local, chan]
        pc_gri = pc_cols.transpose(1, 0, 2).reshape(8, 4, L, D) \
            .transpose(0, 1, 2, 3)                     # (4g+r) major
        # k_local = 4g + r -> index [g, r]: k_local axis is (g*4 + ... wait
        pc_kli = pc_cols.transpose(1, 0, 2)            # [k_local, i, chan]
        pc_gr = pc_kli.reshape(8, 4, L, D)             # [g, r, i, c] k=4g+r
        m["pc"] = np.ascontiguousarray(
            pc_gr.reshape(NTOK, D).T.astype(F16))
        # br[chan, s*256 + j] = biasT[32c + s, j, chan]
        m["br"] = np.ascontiguousarray(
            biasT[32 * c:32 * c + 32].reshape(NTOK, D).T.astype(F16))
        in_maps.append(m)
    return in_maps


def _sharded_fn(nc):
    """Build (once) a cached jitted shard_map callable for the program."""
    if "fn" in _cache:
        return _cache["fn"]
    import jax
    import numpy as _np
    from jax.sharding import Mesh, PartitionSpec
    from jax.experimental.shard_map import shard_map
    from concourse import mybir
    from concourse import bass2jax as b2j

    b2j.install_neuronx_cc_hook()
    pid_name = (nc.partition_id_tensor.name
                if nc.partition_id_tensor is not None else None)
    in_names, out_names, out_shapes, out_dtypes = [], [], [], []
    for alloc in nc.m.functions[0].allocations:
        if not isinstance(alloc, mybir.MemoryLocationSet):
            continue
        name = alloc.memorylocations[0].name
        if alloc.kind == "ExternalInput":
            if name == pid_name:
                continue
            in_names.append(name)
        elif alloc.kind == "ExternalOutput":
            out_names.append(name)
            out_shapes.append(tuple(alloc.tensor_shape))
            out_dtypes.append(mybir.dt.np(alloc.dtype))
    n_params = len(in_names)
    n_outs = len(out_names)
    out_avals = [jax.core.ShapedArray(s, d)
                 for s, d in zip(out_shapes, out_dtypes)]
    all_names = in_names + out_names
    if pid_name is not None:
        all_names = all_names + [pid_name]

    def _body(*args):
        ops = list(args)
        if pid_name is not None:
            ops.append(b2j.partition_id_tensor())
        outs = b2j._bass_exec_p.bind(
            *ops,
            out_avals=tuple(out_avals),
            in_names=tuple(all_names),
            out_names=tuple(out_names),
            lowering_input_output_aliases=(),
            sim_require_finite=True,
            sim_require_nnan=True,
            nc=nc,
        )
        return tuple(outs)

    devices = jax.devices()[:N_CORES]
    mesh = Mesh(_np.asarray(devices), ("core",))
    in_specs = (PartitionSpec("core"),) * (n_params + n_outs)
    out_specs = (PartitionSpec("core"),) * n_outs
    donate = tuple(range(n_params, n_params + n_outs))
    fn = jax.jit(
        shard_map(_body, mesh=mesh, in_specs=in_specs, out_specs=out_specs,
                  check_rep=False),
        donate_argnums=donate, keep_unused=True)
    _cache["fn"] = (fn, in_names, out_names, out_shapes, out_dtypes)
    return _cache["fn"]


def kernel(**inputs):
    nc = _build()
    in_maps = _prep_inputs(inputs)
    fn, in_names, out_names, out_shapes, out_dtypes = _sharded_fn(nc)
    concat_in = [np.concatenate([in_maps[c][n] for c in range(N_CORES)],
                                axis=0)
                 for n in in_names]
    concat_zeros = [np.zeros((N_CORES * s[0], *s[1:]), d)
                    for s, d in zip(out_shapes, out_dtypes)]
    out_arrs = fn(*concat_in, *concat_zeros)
    oc_all = np.asarray(out_arrs[out_names.index("out")]) \
        .reshape(N_CORES, D, NTOK).astype(np.float32)
    out = np.empty((1, L, L, D), dtype=np.float32)
    for c in range(N_CORES):
        # col = g*1024 + kl*256 + i ; k_local = 4g + kl
        oc = oc_all[c].reshape(D, 8, 4, L)       # [D, g, kl, i]
        out[0, 32 * c:32 * c + 32] = \
            oc.transpose(1, 2, 3, 0).reshape(NS, L, D)
    return out


if __name__ == "__main__":
    _build()
    print("build ok")
